# revision 1
# baseline (speedup 1.0000x reference)
"""Link-predictor GNN kernel for 8 TRN2 NeuronCores.

Strategy (per sharding hint): shard edges across 8 cores (data parallel),
replicate the bf16-cast node-embedding table + MLP weights on every core.

Per core (75264 edges = 147 tiles x 512 edges, 21 gather-chunks x 3584):
  1. SWDGE indirect gather: 3584 embedding rows/call (bf16, 256B rows),
     landing [128 lanes, 28 subtiles x 128 d] in SBUF.
  2. PE transpose (bf16, via identity) each [128e,128d] subtile into PSUM
     -> X^T layout [128 d, 512 e]; DVE copies PSUM->SBUF.
  3. matmul1: h[128h, 512e] (2 halves) = W1_blk^T . X^T, K=2x128 accum.
  4. ACT: relu(h + b1) -> bf16 SBUF.
  5. matmul2: logits[1, 512] = W2_blk^T . h, K=2x128 accum.
  6. ACT: sigmoid(logits + b2) -> f32 SBUF; HWDGE DMA to DRAM out.
"""

import os
import sys

sys.path.insert(0, "/opt/trn_rl_repo")

import numpy as np
import ml_dtypes

from concourse import bacc, mybir, tile
from concourse.bass import IndirectOffsetOnAxis
from concourse.bass_utils import run_bass_kernel_spmd

BF16 = ml_dtypes.bfloat16

N_NODES = 100000
D = 128
H = 256
E_TOTAL = 600000
NCORES = 8
E_CORE = 75000          # real edges per core
TILE_E = 512            # edges per compute tile
TILES_PER_CHUNK = 7
SUB = 4 * TILES_PER_CHUNK          # 28 gather subtiles (128 edges) per chunk
CHUNK_E = SUB * 128                # 3584 edges per gather chunk
CHUNKS = 21
EC_PAD = CHUNKS * CHUNK_E          # 75264 padded edges per core
NT = CHUNKS * TILES_PER_CHUNK      # 147 tiles

LAST_RESULTS = None
_NC = None


def _build_program():
    global _NC
    if _NC is not None:
        return _NC
    dt = mybir.dt
    nc = bacc.Bacc(
        "TRN2",
        target_bir_lowering=False,
        debug=False,
        enable_asserts=False,
        num_devices=NCORES,
    )
    emd = nc.dram_tensor("emd", [N_NODES, D], dt.bfloat16, kind="ExternalInput")
    soff_d = nc.dram_tensor("soff", [128, CHUNKS * SUB], dt.int32, kind="ExternalInput")
    doff_d = nc.dram_tensor("doff", [128, CHUNKS * SUB], dt.int32, kind="ExternalInput")
    w1_d = nc.dram_tensor("w1", [128, 512], dt.bfloat16, kind="ExternalInput")
    w2_d = nc.dram_tensor("w2", [128, 2], dt.bfloat16, kind="ExternalInput")
    b1_d = nc.dram_tensor("b1", [128, 2], dt.float32, kind="ExternalInput")
    b2_d = nc.dram_tensor("b2", [1, 1], dt.float32, kind="ExternalInput")
    ident_d = nc.dram_tensor("ident", [128, 128], dt.bfloat16, kind="ExternalInput")
    out_d = nc.dram_tensor("out", [NT, TILE_E], dt.float32, kind="ExternalOutput")

    AF = mybir.ActivationFunctionType

    with tile.TileContext(nc) as tc:
        with (
            tc.tile_pool(name="const", bufs=1) as cpool,
            tc.tile_pool(name="g", bufs=2) as gpool,
            tc.tile_pool(name="x", bufs=3) as xpool,
            tc.tile_pool(name="h", bufs=3) as hpool,
            tc.tile_pool(name="o", bufs=4) as opool,
            tc.tile_pool(name="px", bufs=2, space="PSUM") as pxp,
            tc.tile_pool(name="ph", bufs=2, space="PSUM") as php,
            tc.tile_pool(name="pl", bufs=2, space="PSUM") as plp,
        ):
            w1_sb = cpool.tile([128, 512], dt.bfloat16)
            nc.sync.dma_start(w1_sb[:, :], w1_d[:, :])
            w2_sb = cpool.tile([128, 2], dt.bfloat16)
            nc.sync.dma_start(w2_sb[:, :], w2_d[:, :])
            b1_sb = cpool.tile([128, 2], dt.float32)
            nc.sync.dma_start(b1_sb[:, :], b1_d[:, :])
            b2_sb = cpool.tile([1, 1], dt.float32)
            nc.sync.dma_start(b2_sb[:, :], b2_d[:, :])
            ident = cpool.tile([128, 128], dt.bfloat16)
            nc.sync.dma_start(ident[:, :], ident_d[:, :])
            soff = cpool.tile([128, CHUNKS * SUB], dt.int32)
            nc.sync.dma_start(soff[:, :], soff_d[:, :])
            doff = cpool.tile([128, CHUNKS * SUB], dt.int32)
            nc.sync.dma_start(doff[:, :], doff_d[:, :])

            for c in range(CHUNKS):
                g_s = gpool.tile([128, CHUNK_E], dt.bfloat16, tag="gs")
                g_d = gpool.tile([128, CHUNK_E], dt.bfloat16, tag="gd")
                # HW walrus indirect DMA consumes exactly one index per
                # partition (128 rows/call) — one call per 128-edge subtile.
                for m in range(SUB):
                    col = c * SUB + m
                    nc.gpsimd.indirect_dma_start(
                        out=g_s[:, m * 128 : (m + 1) * 128],
                        out_offset=None,
                        in_=emd[:, :],
                        in_offset=IndirectOffsetOnAxis(
                            ap=soff[:, col : col + 1], axis=0
                        ),
                    )
                    nc.gpsimd.indirect_dma_start(
                        out=g_d[:, m * 128 : (m + 1) * 128],
                        out_offset=None,
                        in_=emd[:, :],
                        in_offset=IndirectOffsetOnAxis(
                            ap=doff[:, col : col + 1], axis=0
                        ),
                    )
                for t in range(TILES_PER_CHUNK):
                    T = c * TILES_PER_CHUNK + t
                    # transpose 4 src + 4 dst subtiles into one PSUM tile:
                    # cols 0:512 = Xsrc^T, cols 512:1024 = Xdst^T
                    x_ps = pxp.tile([128, 1024], dt.bfloat16, tag="xps")
                    for i in range(4):
                        m = t * 4 + i
                        nc.tensor.transpose(
                            out=x_ps[:, i * 128 : (i + 1) * 128],
                            in_=g_s[:, m * 128 : (m + 1) * 128],
                            identity=ident[:, :],
                        )
                        nc.tensor.transpose(
                            out=x_ps[:, 512 + i * 128 : 512 + (i + 1) * 128],
                            in_=g_d[:, m * 128 : (m + 1) * 128],
                            identity=ident[:, :],
                        )
                    x_sb = xpool.tile([128, 1024], dt.bfloat16, tag="xsb")
                    nc.vector.tensor_copy(out=x_sb[:, :], in_=x_ps[:, :])

                    h0_ps = php.tile([128, 512], dt.float32, tag="h0")
                    h1_ps = php.tile([128, 512], dt.float32, tag="h1")
                    # h = Xsrc @ W1[:128] + Xdst @ W1[128:]
                    nc.tensor.matmul(
                        h0_ps[:, :], lhsT=w1_sb[:, 0:128], rhs=x_sb[:, 0:512],
                        start=True, stop=False,
                    )
                    nc.tensor.matmul(
                        h0_ps[:, :], lhsT=w1_sb[:, 256:384], rhs=x_sb[:, 512:1024],
                        start=False, stop=True,
                    )
                    nc.tensor.matmul(
                        h1_ps[:, :], lhsT=w1_sb[:, 128:256], rhs=x_sb[:, 0:512],
                        start=True, stop=False,
                    )
                    nc.tensor.matmul(
                        h1_ps[:, :], lhsT=w1_sb[:, 384:512], rhs=x_sb[:, 512:1024],
                        start=False, stop=True,
                    )
                    h0_sb = hpool.tile([128, 512], dt.bfloat16, tag="h0sb")
                    h1_sb = hpool.tile([128, 512], dt.bfloat16, tag="h1sb")
                    nc.scalar.activation(
                        h0_sb[:, :], h0_ps[:, :], AF.Relu, bias=b1_sb[:, 0:1]
                    )
                    nc.scalar.activation(
                        h1_sb[:, :], h1_ps[:, :], AF.Relu, bias=b1_sb[:, 1:2]
                    )
                    l_ps = plp.tile([1, TILE_E], dt.float32, tag="lps")
                    nc.tensor.matmul(
                        l_ps[:, :], lhsT=w2_sb[:, 0:1], rhs=h0_sb[:, :],
                        start=True, stop=False,
                    )
                    nc.tensor.matmul(
                        l_ps[:, :], lhsT=w2_sb[:, 1:2], rhs=h1_sb[:, :],
                        start=False, stop=True,
                    )
                    o_sb = opool.tile([1, TILE_E], dt.float32, tag="osb")
                    nc.scalar.activation(
                        o_sb[:, :], l_ps[:, :], AF.Sigmoid, bias=b2_sb[:, 0:1]
                    )
                    nc.sync.dma_start(out_d[T : T + 1, :], o_sb[:, :])

    nc.compile()
    _NC = nc
    return nc


def _arrange_offsets(idx):
    """[EC_PAD] int32 -> [128, CHUNKS*SUB] so that offs[q, c*SUB+m] is the
    node index of edge c*CHUNK_E + m*128 + q."""
    return np.ascontiguousarray(
        idx.reshape(CHUNKS, SUB, 128).transpose(2, 0, 1).reshape(128, CHUNKS * SUB)
    )


def _prepare_inputs(emd_all, edge_index, W1, b1, W2, b2):
    emd_bf = np.ascontiguousarray(np.asarray(emd_all, dtype=np.float32)).astype(BF16)
    ei = np.asarray(edge_index).astype(np.int32)
    W1 = np.asarray(W1, dtype=np.float32)
    W2 = np.asarray(W2, dtype=np.float32)
    b1 = np.asarray(b1, dtype=np.float32).reshape(-1)
    b2 = np.asarray(b2, dtype=np.float32).reshape(-1)

    # lhsT blocks: cols 0:256 = W1[:128,:] (src side), 256:512 = W1[128:,:]
    w1_arr = np.concatenate([W1[:D, :], W1[D:, :]], axis=1).astype(BF16)
    w2_arr = np.stack([W2[:128, 0], W2[128:, 0]], axis=1).astype(BF16)
    b1_arr = np.ascontiguousarray(np.stack([b1[:128], b1[128:]], axis=1))
    b2_arr = b2.reshape(1, 1)
    ident = np.eye(128, dtype=np.float32).astype(BF16)

    in_maps = []
    for c in range(NCORES):
        sl = ei[c * E_CORE : (c + 1) * E_CORE]
        src = np.zeros(EC_PAD, np.int32)
        dst = np.zeros(EC_PAD, np.int32)
        src[: E_CORE] = sl[:, 0]
        dst[: E_CORE] = sl[:, 1]
        in_maps.append(
            {
                "emd": emd_bf,
                "soff": _arrange_offsets(src),
                "doff": _arrange_offsets(dst),
                "w1": w1_arr,
                "w2": w2_arr,
                "b1": b1_arr,
                "b2": b2_arr,
                "ident": ident,
            }
        )
    return in_maps


def kernel(emd_all, edge_index, W1, b1, W2, b2):
    global LAST_RESULTS
    in_maps = _prepare_inputs(emd_all, edge_index, W1, b1, W2, b2)
    nc = _build_program()
    res = run_bass_kernel_spmd(nc, in_maps, core_ids=list(range(NCORES)))
    LAST_RESULTS = res
    outs = [
        np.asarray(res.results[c]["out"], dtype=np.float32).reshape(-1)[:E_CORE]
        for c in range(NCORES)
    ]
    return np.concatenate(outs).reshape(E_TOTAL, 1)


if __name__ == "__main__":
    rng = np.random.default_rng(0)
    emd = rng.standard_normal((N_NODES, D), dtype=np.float32)
    ei = rng.integers(0, N_NODES, size=(E_TOTAL, 2)).astype(np.int32)
    W1 = rng.standard_normal((2 * D, H), dtype=np.float32) / np.sqrt(2 * D)
    W2 = rng.standard_normal((H, 1), dtype=np.float32) / np.sqrt(H)
    out = kernel(emd, ei, W1, np.zeros(H, np.float32), W2, np.zeros(1, np.float32))
    print(out.shape, out[:4, 0])



# revision 4
# speedup vs baseline: 4.0077x; 4.0077x over previous
"""Link-predictor GNN kernel for 8 TRN2 NeuronCores.

Strategy (per sharding hint): shard edges across 8 cores (data parallel),
replicate the bf16 node-embedding table + MLP weights on every core.

Per core (75000 edges):
  Edges are bucketed host-side by (src_window, dst_window) where a window is
  32768 table rows (4 windows cover 100000 nodes) so node ids fit the int16
  indices of the batched SWDGE dma_gather. One gather per bucket per side
  (transpose=True, single_packet=False) lands embeddings directly in
  X^T layout [128 dims, n_edges] in SBUF — no PE transposes needed and the
  ~1us SWDGE fixed overhead is amortized over ~5000 rows instead of 128.

  MLP per 512-edge tile: h = relu(W1s^T Xs + W1d^T Xd + b1) via 4 matmuls
  accumulating in PSUM; relu of h-half-0 on ACT (bias fused), half-1 on DVE
  (tensor_scalar add+max). Layer 2 contracts h against W2 using h-subtiles
  as the stationary operand: 2 matmuls of N=1 per 128-edge subtile writing
  one PSUM column [128 edges, 1]; a whole bucket's logits accumulate into
  one PSUM tile so a single sigmoid + one small DMA per bucket emits
  [128, cols] f32 results. Host inverts the bucket permutation.
"""

import sys

sys.path.insert(0, "/opt/trn_rl_repo")

import numpy as np
import ml_dtypes

from concourse import bacc, mybir, tile
from concourse.bass_utils import run_bass_kernel_spmd

BF16 = ml_dtypes.bfloat16

N_NODES = 100000
D = 128
H = 256
E_TOTAL = 600000
NCORES = 8
E_CORE = 75000
WIN = 32768                      # int16-addressable window of table rows
NBUCK = 16                       # 4 src windows x 4 dst windows

# Bucket capacities for the canonical setup_inputs() edge set (max count over
# the 8 cores per bucket, rounded up to 128). kernel() recomputes these from
# its actual inputs; this default only serves _build_program() callers that
# have no inputs (e.g. a standalone TimelineSim of the program).
DEFAULT_CAPS = (8192, 8320, 8320, 512, 8192, 8320, 8320, 512,
                8192, 8192, 8192, 512, 512, 512, 512, 128)

LAST_RESULTS = None
_NC_CACHE: dict = {}


def _window(w):
    base = w * WIN
    return base, min(WIN, N_NODES - base)


def _build_program(caps=None):
    if caps is None:
        caps = DEFAULT_CAPS
    caps = tuple(int(c) for c in caps)
    if caps in _NC_CACHE:
        return _NC_CACHE[caps]

    dt = mybir.dt
    AF = mybir.ActivationFunctionType
    ALU = mybir.AluOpType

    EP = sum(caps)
    TOT = EP // 128
    CMAX = max(caps)

    nc = bacc.Bacc(
        "TRN2",
        target_bir_lowering=False,
        debug=False,
        enable_asserts=False,
        num_devices=NCORES,
    )
    emd = nc.dram_tensor("emd", [N_NODES, D], dt.bfloat16, kind="ExternalInput")
    sidx_d = nc.dram_tensor("sidx", [128, EP // 16], dt.int16, kind="ExternalInput")
    didx_d = nc.dram_tensor("didx", [128, EP // 16], dt.int16, kind="ExternalInput")
    w1_d = nc.dram_tensor("w1", [128, 512], dt.bfloat16, kind="ExternalInput")
    b1_d = nc.dram_tensor("b1", [128, 2], dt.float32, kind="ExternalInput")
    w2_d = nc.dram_tensor("w2", [128, 2], dt.bfloat16, kind="ExternalInput")
    b2_d = nc.dram_tensor("b2", [128, 1], dt.float32, kind="ExternalInput")
    out_d = nc.dram_tensor("out", [128, TOT], dt.float32, kind="ExternalOutput")

    with tile.TileContext(nc) as tc:
        with (
            tc.tile_pool(name="const", bufs=1) as cpool,
            tc.tile_pool(name="x", bufs=2) as xpool,
            tc.tile_pool(name="h", bufs=3) as hpool,
            tc.tile_pool(name="o", bufs=2) as opool,
            tc.tile_pool(name="ph", bufs=2, space="PSUM") as php,
            tc.tile_pool(name="pl", bufs=2, space="PSUM") as plp,
        ):
            w1_sb = cpool.tile([128, 512], dt.bfloat16)
            nc.sync.dma_start(w1_sb[:, :], w1_d[:, :])
            b1_sb = cpool.tile([128, 2], dt.float32)
            nc.sync.dma_start(b1_sb[:, :], b1_d[:, :])
            w2_sb = cpool.tile([128, 2], dt.bfloat16)
            nc.sync.dma_start(w2_sb[:, :], w2_d[:, :])
            b2_sb = cpool.tile([128, 1], dt.float32)
            nc.sync.dma_start(b2_sb[:, :], b2_d[:, :])
            sidx_sb = cpool.tile([128, EP // 16], dt.int16)
            nc.sync.dma_start(sidx_sb[:, :], sidx_d[:, :])
            didx_sb = cpool.tile([128, EP // 16], dt.int16)
            nc.sync.dma_start(didx_sb[:, :], didx_d[:, :])

            col16 = 0
            col128 = 0
            for b in range(NBUCK):
                Cb = caps[b]
                ncols = Cb // 128
                sb_, sl_ = _window(b >> 2)
                db_, dl_ = _window(b & 3)
                xs = xpool.tile([128, CMAX], dt.bfloat16, tag="xs")
                xd = xpool.tile([128, CMAX], dt.bfloat16, tag="xd")
                nc.gpsimd.dma_gather(
                    xs[:, 0:Cb].unsqueeze(1),
                    emd[sb_ : sb_ + sl_, :],
                    sidx_sb[:, col16 : col16 + Cb // 16],
                    Cb,
                    Cb,
                    D,
                    transpose=True,
                    single_packet=False,
                )
                nc.gpsimd.dma_gather(
                    xd[:, 0:Cb].unsqueeze(1),
                    emd[db_ : db_ + dl_, :],
                    didx_sb[:, col16 : col16 + Cb // 16],
                    Cb,
                    Cb,
                    D,
                    transpose=True,
                    single_packet=False,
                )

                lg = plp.tile([128, CMAX // 128], dt.float32, tag="lg")
                ntile = (Cb + 511) // 512
                # software pipeline: L2 of tile t issues after L1 of tile t+1
                hq = []
                for t in range(ntile + 1):
                    if t < ntile:
                        e0 = t * 512
                        n = min(512, Cb - e0)
                        h0p = php.tile([128, 512], dt.float32, tag="h0p")
                        h1p = php.tile([128, 512], dt.float32, tag="h1p")
                        nc.tensor.matmul(
                            h0p[:, 0:n], lhsT=w1_sb[:, 0:128],
                            rhs=xs[:, e0 : e0 + n], start=True, stop=False,
                        )
                        nc.tensor.matmul(
                            h0p[:, 0:n], lhsT=w1_sb[:, 256:384],
                            rhs=xd[:, e0 : e0 + n], start=False, stop=True,
                        )
                        nc.tensor.matmul(
                            h1p[:, 0:n], lhsT=w1_sb[:, 128:256],
                            rhs=xs[:, e0 : e0 + n], start=True, stop=False,
                        )
                        nc.tensor.matmul(
                            h1p[:, 0:n], lhsT=w1_sb[:, 384:512],
                            rhs=xd[:, e0 : e0 + n], start=False, stop=True,
                        )
                        h0s = hpool.tile([128, 512], dt.bfloat16, tag="h0s")
                        h1s = hpool.tile([128, 512], dt.bfloat16, tag="h1s")
                        nc.scalar.activation(
                            h0s[:, 0:n], h0p[:, 0:n], AF.Relu, bias=b1_sb[:, 0:1]
                        )
                        nc.vector.tensor_scalar(
                            h1s[:, 0:n], h1p[:, 0:n],
                            b1_sb[:, 1:2], 0.0, ALU.add, ALU.max,
                        )
                        hq.append((t, n, h0s, h1s))
                    if t >= 1:
                        pt, pn, p0, p1 = hq[t - 1]
                        for s in range((pn + 127) // 128):
                            ns = min(128, pn - s * 128)
                            col = pt * 4 + s
                            nc.tensor.matmul(
                                lg[0:ns, col : col + 1],
                                lhsT=p0[:, s * 128 : s * 128 + ns],
                                rhs=w2_sb[:, 0:1], start=True, stop=False,
                            )
                            nc.tensor.matmul(
                                lg[0:ns, col : col + 1],
                                lhsT=p1[:, s * 128 : s * 128 + ns],
                                rhs=w2_sb[:, 1:2], start=False, stop=True,
                            )
                sig = opool.tile([128, CMAX // 128], dt.float32, tag="sig")
                nc.scalar.activation(
                    sig[:, 0:ncols], lg[:, 0:ncols], AF.Sigmoid, bias=b2_sb[:, 0:1]
                )
                nc.sync.dma_start(out_d[:, col128 : col128 + ncols], sig[:, 0:ncols])
                col16 += Cb // 16
                col128 += ncols

    nc.compile()
    _NC_CACHE[caps] = nc
    return nc


def _wrap_idx(vals):
    """int16 [n] -> [128, n//16] wrapped in 16 partitions, replicated x8."""
    n = vals.shape[0]
    return np.tile(vals.reshape(n // 16, 16).T, (8, 1))


def _prepare_core(ei_core, caps):
    """Bucket one core's edges; returns (sidx, didx, order, counts)."""
    src = ei_core[:, 0].astype(np.int64)
    dst = ei_core[:, 1].astype(np.int64)
    bucket = (src >> 15) * 4 + (dst >> 15)
    order = np.argsort(bucket, kind="stable")
    counts = np.bincount(bucket, minlength=NBUCK)
    EP = sum(caps)
    sidx = np.zeros(EP, np.int16)
    didx = np.zeros(EP, np.int16)
    ofs = 0
    base = 0
    for b in range(NBUCK):
        nb = int(counts[b])
        sel = order[ofs : ofs + nb]
        sidx[base : base + nb] = (src[sel] - ((b >> 2) << 15)).astype(np.int16)
        didx[base : base + nb] = (dst[sel] - ((b & 3) << 15)).astype(np.int16)
        ofs += nb
        base += caps[b]
    return _wrap_idx(sidx), _wrap_idx(didx), order, counts


def kernel(emd_all, edge_index, W1, b1, W2, b2):
    global LAST_RESULTS
    emd_bf = np.ascontiguousarray(np.asarray(emd_all, dtype=np.float32)).astype(BF16)
    ei = np.asarray(edge_index).astype(np.int64)
    W1 = np.asarray(W1, dtype=np.float32)
    W2 = np.asarray(W2, dtype=np.float32)
    b1 = np.asarray(b1, dtype=np.float32).reshape(-1)
    b2 = np.asarray(b2, dtype=np.float32).reshape(-1)

    # per-(core,bucket) counts -> shared static capacities
    all_counts = np.zeros((NCORES, NBUCK), np.int64)
    for c in range(NCORES):
        el = ei[c * E_CORE : (c + 1) * E_CORE]
        bk = (el[:, 0] >> 15) * 4 + (el[:, 1] >> 15)
        all_counts[c] = np.bincount(bk, minlength=NBUCK)
    caps = tuple(
        int(max(128, -(-int(all_counts[:, b].max()) // 128) * 128))
        for b in range(NBUCK)
    )

    # lhsT blocks: [src->h0, src->h1, dst->h0, dst->h1]
    w1_arr = np.concatenate(
        [W1[:D, :D], W1[:D, D:], W1[D:, :D], W1[D:, D:]], axis=1
    ).astype(BF16)
    b1_arr = np.ascontiguousarray(np.stack([b1[:128], b1[128:]], axis=1))
    w2_arr = np.ascontiguousarray(np.stack([W2[:128, 0], W2[128:, 0]], axis=1)).astype(
        BF16
    )
    b2_arr = np.full((128, 1), b2[0], np.float32)

    in_maps = []
    unshard = []
    for c in range(NCORES):
        sidx, didx, order, counts = _prepare_core(
            ei[c * E_CORE : (c + 1) * E_CORE], caps
        )
        unshard.append((order, counts))
        in_maps.append(
            {
                "emd": emd_bf,
                "sidx": sidx,
                "didx": didx,
                "w1": w1_arr,
                "b1": b1_arr,
                "w2": w2_arr,
                "b2": b2_arr,
            }
        )

    nc = _build_program(caps)
    res = run_bass_kernel_spmd(nc, in_maps, core_ids=list(range(NCORES)))
    LAST_RESULTS = res

    y = np.empty((E_TOTAL,), np.float32)
    for c in range(NCORES):
        order, counts = unshard[c]
        out = np.asarray(res.results[c]["out"], dtype=np.float32)  # [128, TOT]
        yc = np.empty((E_CORE,), np.float32)
        ofs = 0
        cb = 0
        for b in range(NBUCK):
            nb = int(counts[b])
            ncols = caps[b] // 128
            block = out[:, cb : cb + ncols].T.reshape(-1)[:nb]
            yc[order[ofs : ofs + nb]] = block
            ofs += nb
            cb += ncols
        y[c * E_CORE : (c + 1) * E_CORE] = yc
    return y.reshape(E_TOTAL, 1)


if __name__ == "__main__":
    rng = np.random.default_rng(0)
    emd = rng.standard_normal((N_NODES, D), dtype=np.float32)
    ei = rng.integers(0, N_NODES, size=(E_TOTAL, 2)).astype(np.int32)
    W1 = rng.standard_normal((2 * D, H), dtype=np.float32) / np.sqrt(2 * D)
    W2 = rng.standard_normal((H, 1), dtype=np.float32) / np.sqrt(H)
    out = kernel(emd, ei, W1, np.zeros(H, np.float32), W2, np.zeros(1, np.float32))
    print(out.shape, out[:4, 0])


# revision 17
# speedup vs baseline: 5.2017x; 1.2979x over previous
"""Link-predictor GNN kernel for 8 TRN2 NeuronCores.

Strategy (per sharding hint): shard edges across 8 cores (data parallel),
replicate the bf16 node-embedding table + MLP weights on every core.

Per core (75000 edges):
  Edges are bucketed host-side by (src_window, dst_window) where a window is
  32768 table rows (4 windows cover 100000 nodes) so node ids fit the int16
  indices of the batched SWDGE dma_gather. Buckets are cut into <=2048-edge
  chunks for fine-grained pipelining. One gather per chunk per side
  (transpose=True, single_packet=False) lands embeddings directly in
  X^T layout [128 dims, n_edges] in SBUF — no PE transposes needed and the
  ~1us SWDGE fixed overhead is amortized over thousands of rows instead
  of 128.

  MLP per 512-edge tile: h = relu(W1s^T Xs + W1d^T Xd + b1) via 4 matmuls
  accumulating in PSUM; relu of h-half-0 on ACT (bias fused), half-1 on DVE
  (tensor_scalar add+max). Layer 2 contracts h against W2 using h-subtiles
  as the stationary operand: 2 matmuls of N=1 per 128-edge subtile writing
  one PSUM column [128 edges, 1]; a whole chunk's logits accumulate into
  one PSUM tile so a single sigmoid + one small DMA per chunk emits
  [128, cols] f32 results. Host inverts the slot permutation.
"""

import sys

sys.path.insert(0, "/opt/trn_rl_repo")

import numpy as np
import ml_dtypes

from concourse import bacc, mybir, tile
from concourse.bass_utils import run_bass_kernel_spmd

BF16 = ml_dtypes.bfloat16

N_NODES = 100000
D = 128
H = 256
E_TOTAL = 600000
NCORES = 8
E_CORE = 75000
WIN = 25000                      # table-row window (< 2^15 for int16 idx)
NBUCK = 16                       # 4 src windows x 4 dst windows
CHUNK = 2688                     # max edges per gather/compute chunk
XBUFS = 4                        # gather buffer depth per side
IBUFS = 4                        # idx tile depth
HPBUFS = 2                       # PSUM h depth
L2LAG = 1                        # tiles of lag between L1 and L2 issue
HBUFS = 3                        # h sbuf tile depth
PLAN_MODE = "smalls_last"        # or "interleave"

# Bucket capacities for the canonical setup_inputs() edge set (max count over
# the 8 cores per bucket, rounded up to 128). kernel() recomputes these from
# its actual inputs; this default only serves _build_program() callers that
# have no inputs (e.g. a standalone TimelineSim of the program).
DEFAULT_CAPS = (4736, 4736, 4736, 4864, 4736, 4736, 4736, 4736,
                4736, 4736, 4736, 4736, 4736, 4736, 4736, 4736)

LAST_RESULTS = None
_NC_CACHE: dict = {}


def _window(w):
    base = w * WIN
    return base, min(WIN, N_NODES - base)


def _bucket_of(src, dst):
    return (src // WIN) * 4 + dst // WIN


def _chunk_plan(caps):
    """Cut buckets into chunks and pick a processing order.

    Returns a list of (bucket, offset_in_bucket, size, slot_base) with
    slot_base assigned in processing order. Order: smallest chunk first
    (fast pipeline fill), small chunks spread evenly among the big ones
    (their SWDGE descriptor-gen overhead hides under big transfers), and a
    small chunk last (short drain tail).
    """
    chunks = []
    for b in range(NBUCK):
        o = 0
        while o < caps[b]:
            sz = min(CHUNK, caps[b] - o)
            chunks.append((b, o, sz))
            o += sz
    chunks.sort(key=lambda c: -c[2])
    bigs = [c for c in chunks if c[2] >= CHUNK]
    smalls = [c for c in chunks if c[2] < CHUNK]
    seq = []
    first = smalls.pop() if smalls else (bigs.pop() if bigs else None)
    if first:
        seq.append(first)
    if PLAN_MODE == "interleave" and smalls and bigs:
        last = smalls.pop() if smalls else None
        stride = -(-len(bigs) // (len(smalls) + 1))
        bi = 0
        si = 0
        while bi < len(bigs):
            seq.extend(bigs[bi : bi + stride])
            bi += stride
            if si < len(smalls):
                seq.append(smalls[si])
                si += 1
        seq.extend(smalls[si:])
        if last:
            seq.append(last)
    else:
        seq.extend(bigs)
        seq.extend(smalls)
    plan = []
    base = 0
    for b, o, sz in seq:
        plan.append((b, o, sz, base))
        base += sz
    return plan


def _build_program(caps=None):
    if caps is None:
        caps = DEFAULT_CAPS
    caps = tuple(int(c) for c in caps)
    if caps in _NC_CACHE:
        return _NC_CACHE[caps]

    dt = mybir.dt
    AF = mybir.ActivationFunctionType
    ALU = mybir.AluOpType

    EP = sum(caps)
    TOT = EP // 128
    plan = _chunk_plan(caps)
    CMAX = max(sz for _, _, sz, _ in plan)

    nc = bacc.Bacc(
        "TRN2",
        target_bir_lowering=False,
        debug=False,
        enable_asserts=False,
        num_devices=NCORES,
    )
    emd = nc.dram_tensor("emd", [N_NODES, D], dt.bfloat16, kind="ExternalInput")
    idx_d = nc.dram_tensor("idx", [128, EP // 8], dt.int16, kind="ExternalInput")
    w1_d = nc.dram_tensor("w1", [128, 512], dt.bfloat16, kind="ExternalInput")
    b1_d = nc.dram_tensor("b1", [128, 2], dt.float32, kind="ExternalInput")
    w2_d = nc.dram_tensor("w2", [128, 2], dt.bfloat16, kind="ExternalInput")
    b2_d = nc.dram_tensor("b2", [128, 1], dt.float32, kind="ExternalInput")
    out_d = nc.dram_tensor("out", [128, TOT], dt.float32, kind="ExternalOutput")

    with tile.TileContext(nc) as tc:
        with (
            tc.tile_pool(name="const", bufs=1) as cpool,
            tc.tile_pool(name="i", bufs=IBUFS) as ipool,
            tc.tile_pool(name="x", bufs=XBUFS) as xpool,
            tc.tile_pool(name="h", bufs=HBUFS) as hpool,
            tc.tile_pool(name="o", bufs=2) as opool,
            tc.tile_pool(name="ph", bufs=HPBUFS, space="PSUM") as php,
            tc.tile_pool(name="pl", bufs=2, space="PSUM") as plp,
        ):
            # weight loads ride the Activation engine's HWDGE queue so the
            # first chunks' idx loads lead the SP queue
            w1_sb = cpool.tile([128, 512], dt.bfloat16)
            nc.scalar.dma_start(w1_sb[:, :], w1_d[:, :])
            b1_sb = cpool.tile([128, 2], dt.float32)
            nc.scalar.dma_start(b1_sb[:, :], b1_d[:, :])
            w2_sb = cpool.tile([128, 2], dt.bfloat16)
            nc.scalar.dma_start(w2_sb[:, :], w2_d[:, :])
            b2_sb = cpool.tile([128, 1], dt.float32)
            nc.scalar.dma_start(b2_sb[:, :], b2_d[:, :])

            for b, o, sz, base in plan:
                c8 = base // 8
                ncols = sz // 128
                sb_, sl_ = _window(b >> 2)
                db_, dl_ = _window(b & 3)
                sdi = ipool.tile([128, CMAX // 8], dt.int16, tag="sdi")
                nc.sync.dma_start(
                    sdi[:, 0 : sz // 8], idx_d[:, c8 : c8 + sz // 8]
                )
                si = sdi[:, 0 : sz // 16]
                di = sdi[:, sz // 16 : sz // 8]
                xs = xpool.tile([128, CMAX], dt.bfloat16, tag="xs")
                xd = xpool.tile([128, CMAX], dt.bfloat16, tag="xd")
                nc.gpsimd.dma_gather(
                    xs[:, 0:sz].unsqueeze(1),
                    emd[sb_ : sb_ + sl_, :],
                    si,
                    sz,
                    sz,
                    D,
                    transpose=True,
                    single_packet=False,
                )
                nc.gpsimd.dma_gather(
                    xd[:, 0:sz].unsqueeze(1),
                    emd[db_ : db_ + dl_, :],
                    di,
                    sz,
                    sz,
                    D,
                    transpose=True,
                    single_packet=False,
                )

                lg = plp.tile([128, CMAX // 128], dt.float32, tag="lg")
                ntile = (sz + 511) // 512
                # software pipeline: L2 of tile t issues L2LAG tiles late
                hq = []
                for t in range(ntile + L2LAG):
                    if t < ntile:
                        e0 = t * 512
                        n = min(512, sz - e0)
                        h0p = php.tile([128, 512], dt.float32, tag="h0p")
                        h1p = php.tile([128, 512], dt.float32, tag="h1p")
                        nc.tensor.matmul(
                            h0p[:, 0:n], lhsT=w1_sb[:, 0:128],
                            rhs=xs[:, e0 : e0 + n], start=True, stop=False,
                        )
                        nc.tensor.matmul(
                            h0p[:, 0:n], lhsT=w1_sb[:, 256:384],
                            rhs=xd[:, e0 : e0 + n], start=False, stop=True,
                        )
                        nc.tensor.matmul(
                            h1p[:, 0:n], lhsT=w1_sb[:, 128:256],
                            rhs=xs[:, e0 : e0 + n], start=True, stop=False,
                        )
                        nc.tensor.matmul(
                            h1p[:, 0:n], lhsT=w1_sb[:, 384:512],
                            rhs=xd[:, e0 : e0 + n], start=False, stop=True,
                        )
                        h0s = hpool.tile([128, 512], dt.bfloat16, tag="h0s")
                        h1s = hpool.tile([128, 512], dt.bfloat16, tag="h1s")
                        nc.scalar.activation(
                            h0s[:, 0:n], h0p[:, 0:n], AF.Relu, bias=b1_sb[:, 0:1]
                        )
                        nc.vector.tensor_scalar(
                            h1s[:, 0:n], h1p[:, 0:n],
                            b1_sb[:, 1:2], 0.0, ALU.add, ALU.max,
                        )
                        hq.append((t, n, h0s, h1s))
                    if t >= L2LAG:
                        pt, pn, p0, p1 = hq[t - L2LAG]
                        for s in range((pn + 127) // 128):
                            ns = min(128, pn - s * 128)
                            col = pt * 4 + s
                            nc.tensor.matmul(
                                lg[0:ns, col : col + 1],
                                lhsT=p0[:, s * 128 : s * 128 + ns],
                                rhs=w2_sb[:, 0:1], start=True, stop=False,
                            )
                            nc.tensor.matmul(
                                lg[0:ns, col : col + 1],
                                lhsT=p1[:, s * 128 : s * 128 + ns],
                                rhs=w2_sb[:, 1:2], start=False, stop=True,
                            )
                sig = opool.tile([128, CMAX // 128], dt.float32, tag="sig")
                nc.scalar.activation(
                    sig[:, 0:ncols], lg[:, 0:ncols], AF.Sigmoid, bias=b2_sb[:, 0:1]
                )
                nc.sync.dma_start(
                    out_d[:, base // 128 : base // 128 + ncols], sig[:, 0:ncols]
                )

    nc.compile()
    _NC_CACHE[caps] = nc
    return nc


def _wrap_idx(vals):
    """int16 [n] -> [128, n//16] wrapped in 16 partitions, replicated x8."""
    n = vals.shape[0]
    return np.tile(vals.reshape(n // 16, 16).T, (8, 1))


def _prepare_core(ei_core, caps, plan):
    """Bucket + chunk one core's edges into slot order.

    Returns (idx_combined, edge_of_slot): idx_combined is [128, EP//8] int16
    holding, per chunk, the wrapped src indices then the wrapped dst indices;
    edge_of_slot maps slot -> local edge id (or -1 for padding).
    """
    src = ei_core[:, 0].astype(np.int64)
    dst = ei_core[:, 1].astype(np.int64)
    bucket = _bucket_of(src, dst)
    order = np.argsort(bucket, kind="stable")
    counts = np.bincount(bucket, minlength=NBUCK)
    starts = np.zeros(NBUCK + 1, np.int64)
    np.cumsum(counts, out=starts[1:])

    EP = sum(caps)
    idx = np.zeros((128, EP // 8), np.int16)
    edge_of_slot = np.full(EP, -1, np.int64)
    for b, o, sz, base in plan:
        nb = int(counts[b])
        lo = min(o, nb)
        hi = min(o + sz, nb)
        sc = np.zeros(sz, np.int16)
        dc = np.zeros(sz, np.int16)
        if hi > lo:
            sel = order[starts[b] + lo : starts[b] + hi]
            sc[: hi - lo] = (src[sel] - (b >> 2) * WIN).astype(np.int16)
            dc[: hi - lo] = (dst[sel] - (b & 3) * WIN).astype(np.int16)
            edge_of_slot[base : base + hi - lo] = sel
        c8 = base // 8
        idx[:, c8 : c8 + sz // 16] = _wrap_idx(sc)
        idx[:, c8 + sz // 16 : c8 + sz // 8] = _wrap_idx(dc)
    return idx, edge_of_slot


def kernel(emd_all, edge_index, W1, b1, W2, b2):
    global LAST_RESULTS
    emd_bf = np.ascontiguousarray(np.asarray(emd_all, dtype=np.float32)).astype(BF16)
    ei = np.asarray(edge_index).astype(np.int64)
    W1 = np.asarray(W1, dtype=np.float32)
    W2 = np.asarray(W2, dtype=np.float32)
    b1 = np.asarray(b1, dtype=np.float32).reshape(-1)
    b2 = np.asarray(b2, dtype=np.float32).reshape(-1)

    # Re-shard edges across cores so each core gets an equal slice of every
    # bucket: per-core bucket counts become ceil(n_b/8), which minimises the
    # shared static capacity padding.
    bk_all = _bucket_of(ei[:, 0].astype(np.int64), ei[:, 1].astype(np.int64))
    gorder = np.argsort(bk_all, kind="stable")
    gcounts = np.bincount(bk_all, minlength=NBUCK)
    core_of_edge = np.empty(E_TOTAL, np.int64)
    pos = 0
    per_core_n = np.zeros(NCORES, np.int64)
    for b in range(NBUCK):
        nb = int(gcounts[b])
        sel = gorder[pos : pos + nb]
        # deal bucket b round-robin-in-blocks across cores
        q, r = divmod(nb, NCORES)
        o = 0
        for c in range(NCORES):
            take = q + (1 if c < r else 0)
            core_of_edge[sel[o : o + take]] = c
            per_core_n[c] += take
            o += take
        pos += nb
    caps_counts = np.zeros((NCORES, NBUCK), np.int64)
    edges_of_core = [np.where(core_of_edge == c)[0] for c in range(NCORES)]
    for c in range(NCORES):
        bk = bk_all[edges_of_core[c]]
        caps_counts[c] = np.bincount(bk, minlength=NBUCK)
    caps = tuple(
        int(max(128, -(-int(caps_counts[:, b].max()) // 128) * 128))
        for b in range(NBUCK)
    )
    plan = _chunk_plan(caps)

    # lhsT blocks: [src->h0, src->h1, dst->h0, dst->h1]
    w1_arr = np.concatenate(
        [W1[:D, :D], W1[:D, D:], W1[D:, :D], W1[D:, D:]], axis=1
    ).astype(BF16)
    b1_arr = np.ascontiguousarray(np.stack([b1[:128], b1[128:]], axis=1))
    w2_arr = np.ascontiguousarray(np.stack([W2[:128, 0], W2[128:, 0]], axis=1)).astype(
        BF16
    )
    b2_arr = np.full((128, 1), b2[0], np.float32)

    in_maps = []
    unshard = []
    for c in range(NCORES):
        idx, edge_of_slot = _prepare_core(ei[edges_of_core[c]], caps, plan)
        unshard.append(edge_of_slot)
        in_maps.append(
            {
                "emd": emd_bf,
                "idx": idx,
                "w1": w1_arr,
                "b1": b1_arr,
                "w2": w2_arr,
                "b2": b2_arr,
            }
        )

    nc = _build_program(caps)
    res = run_bass_kernel_spmd(nc, in_maps, core_ids=list(range(NCORES)))
    LAST_RESULTS = res

    y = np.empty((E_TOTAL,), np.float32)
    for c in range(NCORES):
        edge_of_slot = unshard[c]  # slot -> index into edges_of_core[c]
        out = np.asarray(res.results[c]["out"], dtype=np.float32)  # [128, TOT]
        flat = out.T.reshape(-1)  # slot-ordered
        mask = edge_of_slot >= 0
        y[edges_of_core[c][edge_of_slot[mask]]] = flat[mask]
    return y.reshape(E_TOTAL, 1)


if __name__ == "__main__":
    rng = np.random.default_rng(0)
    emd = rng.standard_normal((N_NODES, D), dtype=np.float32)
    ei = rng.integers(0, N_NODES, size=(E_TOTAL, 2)).astype(np.int32)
    W1 = rng.standard_normal((2 * D, H), dtype=np.float32) / np.sqrt(2 * D)
    W2 = rng.standard_normal((H, 1), dtype=np.float32) / np.sqrt(H)
    out = kernel(emd, ei, W1, np.zeros(H, np.float32), W2, np.zeros(1, np.float32))
    print(out.shape, out[:4, 0])


# revision 24
# speedup vs baseline: 5.2917x; 1.0173x over previous
"""Link-predictor GNN kernel for 8 TRN2 NeuronCores.

Strategy (per sharding hint): shard edges across 8 cores (data parallel),
replicate the bf16 node-embedding table + MLP weights on every core.

Edges are bucketed by (src_window, dst_window) where a window is 25000
table rows (4 windows cover 100000 nodes) so window-relative node ids fit
the int16 indices of the batched SWDGE dma_gather. Each bucket is dealt
evenly across the 8 cores (so the shared static per-bucket capacity padding
is minimal) and cut into <=CHUNK-edge chunks for pipelining.

Per chunk: one dma_gather per side (transpose=True, single_packet=False)
lands embeddings directly in X^T layout [128 dims, n_edges] in SBUF — no PE
transposes needed, and the ~1us SWDGE fixed overhead is amortized over
thousands of rows instead of the 128 an indirect_dma_start moves. The
16-partition-wrapped index arrays are loaded once in compact [16, cols]
form and replicated to the 128 partitions the gather hardware expects via
0/1-matmul broadcasts of the raw bf16 bit patterns (bit-exact, subnormals
included), cutting idx DMA traffic 8x.

MLP per 512-edge tile: h = relu(W1s^T Xs + W1d^T Xd + b1) via 4 matmuls
accumulating in PSUM; relu of h-half-0 on ACT (bias fused), half-1 on DVE
(tensor_scalar add+max). Layer 2 contracts h against W2 using h-subtiles
as the stationary operand: 2 matmuls of N=1 per 128-edge subtile writing
one PSUM column [128 edges, 1] (Ldweights is free in the cost model); a
whole chunk's logits accumulate into one PSUM tile so a single sigmoid +
one small DMA per chunk emits [128, cols] f32 results. Host inverts the
slot permutation.
"""

import sys

sys.path.insert(0, "/opt/trn_rl_repo")

import numpy as np
import ml_dtypes

from concourse import bacc, mybir, tile
from concourse.bass_utils import run_bass_kernel_spmd

BF16 = ml_dtypes.bfloat16

N_NODES = 100000
D = 128
H = 256
E_TOTAL = 600000
NCORES = 8
E_CORE = 75000
WIN = 25000                      # table-row window (< 2^15 for int16 idx)
NBUCK = 16                       # 4 src windows x 4 dst windows
CHUNK = 2944                     # max edges per gather/compute chunk
XBUFS = 3                        # gather buffer depth per side
IBUFS = 3                        # idx tile depth
HPBUFS = 2                       # PSUM h depth
L2LAG = 1                        # tiles of lag between L1 and L2 issue
HBUFS = 3                        # h sbuf tile depth
PLAN_MODE = "smalls_last"        # or "interleave"
TAPER = 0                        # trailing slots re-split into 512-chunks

# Bucket capacities for the canonical setup_inputs() edge set (max count over
# the 8 cores per bucket, rounded up to 128). kernel() recomputes these from
# its actual inputs; this default only serves _build_program() callers that
# have no inputs (e.g. a standalone TimelineSim of the program).
DEFAULT_CAPS = (4736, 4736, 4736, 4864, 4736, 4736, 4736, 4736,
                4736, 4736, 4736, 4736, 4736, 4736, 4736, 4736)

LAST_RESULTS = None
_NC_CACHE: dict = {}


def _window(w):
    base = w * WIN
    return base, min(WIN, N_NODES - base)


def _bucket_of(src, dst):
    return (src // WIN) * 4 + dst // WIN


def _chunk_plan(caps):
    """Cut buckets into chunks and pick a processing order.

    Returns a list of (bucket, offset_in_bucket, size, slot_base) with
    slot_base assigned in processing order. Order: smallest chunk first
    (fast pipeline fill), small chunks spread evenly among the big ones
    (their SWDGE descriptor-gen overhead hides under big transfers), and a
    small chunk last (short drain tail).
    """
    chunks = []
    for b in range(NBUCK):
        o = 0
        while o < caps[b]:
            sz = min(CHUNK, caps[b] - o)
            chunks.append((b, o, sz))
            o += sz
    chunks.sort(key=lambda c: -c[2])
    bigs = [c for c in chunks if c[2] >= CHUNK]
    smalls = [c for c in chunks if c[2] < CHUNK]
    seq = []
    first = smalls.pop() if smalls else (bigs.pop() if bigs else None)
    if first:
        seq.append(first)
    if PLAN_MODE == "interleave" and smalls and bigs:
        last = smalls.pop() if smalls else None
        stride = -(-len(bigs) // (len(smalls) + 1))
        bi = 0
        si = 0
        while bi < len(bigs):
            seq.extend(bigs[bi : bi + stride])
            bi += stride
            if si < len(smalls):
                seq.append(smalls[si])
                si += 1
        seq.extend(smalls[si:])
        if last:
            seq.append(last)
    else:
        seq.extend(bigs)
        seq.extend(smalls)
    # taper: re-split the trailing slots into 512-edge chunks so the drain
    # after the last big transfer is short
    tail = []
    acc = 0
    while seq and acc < TAPER and seq[-1][2] > 512:
        b, o, sz = seq.pop()
        for i in range(0, sz, 512):
            tail.append((b, o + i, min(512, sz - i)))
        acc += sz
    seq.extend(tail)
    plan = []
    base = 0
    for b, o, sz in seq:
        plan.append((b, o, sz, base))
        base += sz
    return plan


def _build_program(caps=None):
    if caps is None:
        caps = DEFAULT_CAPS
    caps = tuple(int(c) for c in caps)
    if caps in _NC_CACHE:
        return _NC_CACHE[caps]

    dt = mybir.dt
    AF = mybir.ActivationFunctionType
    ALU = mybir.AluOpType

    EP = sum(caps)
    TOT = EP // 128
    plan = _chunk_plan(caps)
    CMAX = max(sz for _, _, sz, _ in plan)

    nc = bacc.Bacc(
        "TRN2",
        target_bir_lowering=False,
        debug=False,
        enable_asserts=False,
        num_devices=NCORES,
    )
    emd = nc.dram_tensor("emd", [N_NODES, D], dt.bfloat16, kind="ExternalInput")
    idx_d = nc.dram_tensor("idx", [16, EP // 8], dt.int16, kind="ExternalInput")
    sel_d = nc.dram_tensor("sel", [16, 128], dt.bfloat16, kind="ExternalInput")
    w1_d = nc.dram_tensor("w1", [128, 512], dt.bfloat16, kind="ExternalInput")
    b1_d = nc.dram_tensor("b1", [128, 2], dt.float32, kind="ExternalInput")
    w2_d = nc.dram_tensor("w2", [128, 2], dt.bfloat16, kind="ExternalInput")
    b2_d = nc.dram_tensor("b2", [128, 1], dt.float32, kind="ExternalInput")
    out_d = nc.dram_tensor("out", [128, TOT], dt.float32, kind="ExternalOutput")

    with tile.TileContext(nc) as tc:
        with (
            tc.tile_pool(name="const", bufs=1) as cpool,
            tc.tile_pool(name="i", bufs=IBUFS) as ipool,
            tc.tile_pool(name="x", bufs=XBUFS) as xpool,
            tc.tile_pool(name="h", bufs=HBUFS) as hpool,
            tc.tile_pool(name="o", bufs=2) as opool,
            tc.tile_pool(name="ph", bufs=HPBUFS, space="PSUM") as php,
            tc.tile_pool(name="pl", bufs=2, space="PSUM") as plp,
            tc.tile_pool(name="pb", bufs=2, space="PSUM") as pbp,
        ):
            # index-path loads go first (the gather pipeline depends on
            # them); weight loads ride the Activation engine's HWDGE queue
            sel_sb = cpool.tile([16, 128], dt.bfloat16)
            nc.sync.dma_start(sel_sb[:, :], sel_d[:, :])
            # all (16-partition-wrapped) gather indices, loaded once; each
            # chunk broadcasts its slice to 128 partitions via a 0/1 matmul
            # on the raw bf16 bit patterns (exact, incl. subnormals)
            idx16_sb = cpool.tile([16, EP // 8], dt.int16)
            nc.sync.dma_start(idx16_sb[:, :], idx_d[:, :])
            w1_sb = cpool.tile([128, 512], dt.bfloat16)
            nc.scalar.dma_start(w1_sb[:, :], w1_d[:, :])
            b1_sb = cpool.tile([128, 2], dt.float32)
            nc.scalar.dma_start(b1_sb[:, :], b1_d[:, :])
            w2_sb = cpool.tile([128, 2], dt.bfloat16)
            nc.scalar.dma_start(w2_sb[:, :], w2_d[:, :])
            b2_sb = cpool.tile([128, 1], dt.float32)
            nc.scalar.dma_start(b2_sb[:, :], b2_d[:, :])

            # broadcast every chunk's indices to 128 partitions up front;
            # PE/DVE pipeline stays far ahead of the gathers consuming them
            sdis = []
            for k, (b, o, sz, base) in enumerate(plan):
                c8 = base // 8
                ibx = pbp.tile([128, CMAX // 8], dt.float32, tag="ibx")
                nc.tensor.matmul(
                    ibx[:, 0 : sz // 8], lhsT=sel_sb[:, :],
                    rhs=idx16_sb[:, c8 : c8 + sz // 8].bitcast(dt.bfloat16),
                    start=True, stop=True,
                )
                sdi = cpool.tile([128, sz // 8], dt.int16, name=f"sdi{k}")
                nc.vector.tensor_copy(
                    out=sdi[:, :].bitcast(dt.bfloat16),
                    in_=ibx[:, 0 : sz // 8],
                )
                sdis.append(sdi)

            for k, (b, o, sz, base) in enumerate(plan):
                ncols = sz // 128
                sb_, sl_ = _window(b >> 2)
                db_, dl_ = _window(b & 3)
                sdi = sdis[k]
                si = sdi[:, 0 : sz // 16]
                di = sdi[:, sz // 16 : sz // 8]
                xs = xpool.tile([128, CMAX], dt.bfloat16, tag="xs")
                xd = xpool.tile([128, CMAX], dt.bfloat16, tag="xd")
                nc.gpsimd.dma_gather(
                    xs[:, 0:sz].unsqueeze(1),
                    emd[sb_ : sb_ + sl_, :],
                    si,
                    sz,
                    sz,
                    D,
                    transpose=True,
                    single_packet=False,
                )
                nc.gpsimd.dma_gather(
                    xd[:, 0:sz].unsqueeze(1),
                    emd[db_ : db_ + dl_, :],
                    di,
                    sz,
                    sz,
                    D,
                    transpose=True,
                    single_packet=False,
                )

                lg = plp.tile([128, CMAX // 128], dt.float32, tag="lg")
                ntile = (sz + 511) // 512
                # software pipeline: L2 of tile t issues L2LAG tiles late
                hq = []
                for t in range(ntile + L2LAG):
                    if t < ntile:
                        e0 = t * 512
                        n = min(512, sz - e0)
                        h0p = php.tile([128, 512], dt.float32, tag="h0p")
                        h1p = php.tile([128, 512], dt.float32, tag="h1p")
                        nc.tensor.matmul(
                            h0p[:, 0:n], lhsT=w1_sb[:, 0:128],
                            rhs=xs[:, e0 : e0 + n], start=True, stop=False,
                        )
                        nc.tensor.matmul(
                            h0p[:, 0:n], lhsT=w1_sb[:, 256:384],
                            rhs=xd[:, e0 : e0 + n], start=False, stop=True,
                        )
                        nc.tensor.matmul(
                            h1p[:, 0:n], lhsT=w1_sb[:, 128:256],
                            rhs=xs[:, e0 : e0 + n], start=True, stop=False,
                        )
                        nc.tensor.matmul(
                            h1p[:, 0:n], lhsT=w1_sb[:, 384:512],
                            rhs=xd[:, e0 : e0 + n], start=False, stop=True,
                        )
                        h0s = hpool.tile([128, 512], dt.bfloat16, tag="h0s")
                        h1s = hpool.tile([128, 512], dt.bfloat16, tag="h1s")
                        nc.scalar.activation(
                            h0s[:, 0:n], h0p[:, 0:n], AF.Relu, bias=b1_sb[:, 0:1]
                        )
                        nc.vector.tensor_scalar(
                            h1s[:, 0:n], h1p[:, 0:n],
                            b1_sb[:, 1:2], 0.0, ALU.add, ALU.max,
                        )
                        hq.append((t, n, h0s, h1s))
                    if t >= L2LAG:
                        pt, pn, p0, p1 = hq[t - L2LAG]
                        for s in range((pn + 127) // 128):
                            ns = min(128, pn - s * 128)
                            col = pt * 4 + s
                            nc.tensor.matmul(
                                lg[0:ns, col : col + 1],
                                lhsT=p0[:, s * 128 : s * 128 + ns],
                                rhs=w2_sb[:, 0:1], start=True, stop=False,
                            )
                            nc.tensor.matmul(
                                lg[0:ns, col : col + 1],
                                lhsT=p1[:, s * 128 : s * 128 + ns],
                                rhs=w2_sb[:, 1:2], start=False, stop=True,
                            )
                sig = opool.tile([128, CMAX // 128], dt.float32, tag="sig")
                nc.scalar.activation(
                    sig[:, 0:ncols], lg[:, 0:ncols], AF.Sigmoid, bias=b2_sb[:, 0:1]
                )
                nc.sync.dma_start(
                    out_d[:, base // 128 : base // 128 + ncols], sig[:, 0:ncols]
                )

    nc.compile()
    _NC_CACHE[caps] = nc
    return nc


def _wrap_idx(vals):
    """int16 [n] -> [16, n//16] wrapped in 16 partitions."""
    n = vals.shape[0]
    return np.ascontiguousarray(vals.reshape(n // 16, 16).T)


def _prepare_core(ei_core, caps, plan):
    """Bucket + chunk one core's edges into slot order.

    Returns (idx_combined, edge_of_slot): idx_combined is [16, EP//8] int16
    holding, per chunk, the wrapped src indices then the wrapped dst indices;
    edge_of_slot maps slot -> local edge id (or -1 for padding).
    """
    src = ei_core[:, 0].astype(np.int64)
    dst = ei_core[:, 1].astype(np.int64)
    bucket = _bucket_of(src, dst)
    order = np.argsort(bucket, kind="stable")
    counts = np.bincount(bucket, minlength=NBUCK)
    starts = np.zeros(NBUCK + 1, np.int64)
    np.cumsum(counts, out=starts[1:])

    EP = sum(caps)
    idx = np.zeros((16, EP // 8), np.int16)
    edge_of_slot = np.full(EP, -1, np.int64)
    for b, o, sz, base in plan:
        nb = int(counts[b])
        lo = min(o, nb)
        hi = min(o + sz, nb)
        sc = np.zeros(sz, np.int16)
        dc = np.zeros(sz, np.int16)
        if hi > lo:
            sel = order[starts[b] + lo : starts[b] + hi]
            sc[: hi - lo] = (src[sel] - (b >> 2) * WIN).astype(np.int16)
            dc[: hi - lo] = (dst[sel] - (b & 3) * WIN).astype(np.int16)
            edge_of_slot[base : base + hi - lo] = sel
        c8 = base // 8
        idx[:, c8 : c8 + sz // 16] = _wrap_idx(sc)
        idx[:, c8 + sz // 16 : c8 + sz // 8] = _wrap_idx(dc)
    return idx, edge_of_slot


def kernel(emd_all, edge_index, W1, b1, W2, b2):
    global LAST_RESULTS
    emd_bf = np.ascontiguousarray(np.asarray(emd_all, dtype=np.float32)).astype(BF16)
    ei = np.asarray(edge_index).astype(np.int64)
    W1 = np.asarray(W1, dtype=np.float32)
    W2 = np.asarray(W2, dtype=np.float32)
    b1 = np.asarray(b1, dtype=np.float32).reshape(-1)
    b2 = np.asarray(b2, dtype=np.float32).reshape(-1)

    # Re-shard edges across cores so each core gets an equal slice of every
    # bucket: per-core bucket counts become ceil(n_b/8), which minimises the
    # shared static capacity padding.
    bk_all = _bucket_of(ei[:, 0].astype(np.int64), ei[:, 1].astype(np.int64))
    gorder = np.argsort(bk_all, kind="stable")
    gcounts = np.bincount(bk_all, minlength=NBUCK)
    core_of_edge = np.empty(E_TOTAL, np.int64)
    pos = 0
    per_core_n = np.zeros(NCORES, np.int64)
    for b in range(NBUCK):
        nb = int(gcounts[b])
        sel = gorder[pos : pos + nb]
        # deal bucket b round-robin-in-blocks across cores
        q, r = divmod(nb, NCORES)
        o = 0
        for c in range(NCORES):
            take = q + (1 if c < r else 0)
            core_of_edge[sel[o : o + take]] = c
            per_core_n[c] += take
            o += take
        pos += nb
    caps_counts = np.zeros((NCORES, NBUCK), np.int64)
    edges_of_core = [np.where(core_of_edge == c)[0] for c in range(NCORES)]
    for c in range(NCORES):
        bk = bk_all[edges_of_core[c]]
        caps_counts[c] = np.bincount(bk, minlength=NBUCK)
    caps = tuple(
        int(max(128, -(-int(caps_counts[:, b].max()) // 128) * 128))
        for b in range(NBUCK)
    )
    plan = _chunk_plan(caps)

    # lhsT blocks: [src->h0, src->h1, dst->h0, dst->h1]
    w1_arr = np.concatenate(
        [W1[:D, :D], W1[:D, D:], W1[D:, :D], W1[D:, D:]], axis=1
    ).astype(BF16)
    b1_arr = np.ascontiguousarray(np.stack([b1[:128], b1[128:]], axis=1))
    w2_arr = np.ascontiguousarray(np.stack([W2[:128, 0], W2[128:, 0]], axis=1)).astype(
        BF16
    )
    b2_arr = np.full((128, 1), b2[0], np.float32)
    sel_arr = np.zeros((16, 128), np.float32)
    sel_arr[np.arange(128) % 16, np.arange(128)] = 1.0
    sel_arr = sel_arr.astype(BF16)

    in_maps = []
    unshard = []
    for c in range(NCORES):
        idx, edge_of_slot = _prepare_core(ei[edges_of_core[c]], caps, plan)
        unshard.append(edge_of_slot)
        in_maps.append(
            {
                "emd": emd_bf,
                "idx": idx,
                "sel": sel_arr,
                "w1": w1_arr,
                "b1": b1_arr,
                "w2": w2_arr,
                "b2": b2_arr,
            }
        )

    nc = _build_program(caps)
    res = run_bass_kernel_spmd(nc, in_maps, core_ids=list(range(NCORES)))
    LAST_RESULTS = res

    y = np.empty((E_TOTAL,), np.float32)
    for c in range(NCORES):
        edge_of_slot = unshard[c]  # slot -> index into edges_of_core[c]
        out = np.asarray(res.results[c]["out"], dtype=np.float32)  # [128, TOT]
        flat = out.T.reshape(-1)  # slot-ordered
        mask = edge_of_slot >= 0
        y[edges_of_core[c][edge_of_slot[mask]]] = flat[mask]
    return y.reshape(E_TOTAL, 1)


if __name__ == "__main__":
    rng = np.random.default_rng(0)
    emd = rng.standard_normal((N_NODES, D), dtype=np.float32)
    ei = rng.integers(0, N_NODES, size=(E_TOTAL, 2)).astype(np.int32)
    W1 = rng.standard_normal((2 * D, H), dtype=np.float32) / np.sqrt(2 * D)
    W2 = rng.standard_normal((H, 1), dtype=np.float32) / np.sqrt(H)
    out = kernel(emd, ei, W1, np.zeros(H, np.float32), W2, np.zeros(1, np.float32))
    print(out.shape, out[:4, 0])


# revision 29
# speedup vs baseline: 5.3079x; 1.0030x over previous
"""Link-predictor GNN kernel for 8 TRN2 NeuronCores.

Strategy (per sharding hint): shard edges across 8 cores (data parallel),
replicate the bf16 node-embedding table + MLP weights on every core.

Edges are bucketed by (src_window, dst_window) where a window is 25000
table rows (4 windows cover 100000 nodes) so window-relative node ids fit
the int16 indices of the batched SWDGE dma_gather. Each bucket is dealt
evenly across the 8 cores (so the shared static per-bucket capacity padding
is minimal) and cut into <=CHUNK-edge chunks for pipelining.

Per chunk: one dma_gather per side (transpose=True, single_packet=False)
lands embeddings directly in X^T layout [128 dims, n_edges] in SBUF — no PE
transposes needed, and the ~1us SWDGE fixed overhead is amortized over
thousands of rows instead of the 128 an indirect_dma_start moves. The
16-partition-wrapped index arrays are loaded once in compact [16, cols]
form and replicated to the 128 partitions the gather hardware expects via
0/1-matmul broadcasts of the raw bf16 bit patterns (bit-exact, subnormals
included), cutting idx DMA traffic 8x.

MLP per 512-edge tile: h = relu(W1s^T Xs + W1d^T Xd + b1) via 4 matmuls
accumulating in PSUM; relu of h-half-0 on ACT (bias fused), half-1 on DVE
(tensor_scalar add+max). Layer 2 contracts h against W2 using h-subtiles
as the stationary operand: 2 matmuls of N=1 per 128-edge subtile writing
one PSUM column [128 edges, 1] (Ldweights is free in the cost model); a
whole chunk's logits accumulate into one PSUM tile so a single sigmoid +
one small DMA per chunk emits [128, cols] f32 results. Host inverts the
slot permutation.
"""

import sys

sys.path.insert(0, "/opt/trn_rl_repo")

import numpy as np
import ml_dtypes

from concourse import bacc, mybir, tile
from concourse.bass_utils import run_bass_kernel_spmd

BF16 = ml_dtypes.bfloat16

N_NODES = 100000
D = 128
H = 256
E_TOTAL = 600000
NCORES = 8
E_CORE = 75000
WIN = 25000                      # table-row window (< 2^15 for int16 idx)
NBUCK = 16                       # 4 src windows x 4 dst windows
CHUNK = 2944                     # max edges per gather/compute chunk
XBUFS = 3                        # gather buffer depth per side
IBUFS = 3                        # idx tile depth
HPBUFS = 2                       # PSUM h depth
L2LAG = 1                        # tiles of lag between L1 and L2 issue
HBUFS = 3                        # h sbuf tile depth
PLAN_MODE = "smalls_last"        # or "interleave"
TAPER = 6000                     # trailing slots re-split into smaller chunks
TAPER_PIECE = 1024               # taper piece size (multiple of 128)

# Bucket capacities for the canonical setup_inputs() edge set (max count over
# the 8 cores per bucket, rounded up to 128). kernel() recomputes these from
# its actual inputs; this default only serves _build_program() callers that
# have no inputs (e.g. a standalone TimelineSim of the program).
DEFAULT_CAPS = (4736, 4736, 4736, 4864, 4736, 4736, 4736, 4736,
                4736, 4736, 4736, 4736, 4736, 4736, 4736, 4736)

LAST_RESULTS = None
_NC_CACHE: dict = {}


def _window(w):
    base = w * WIN
    return base, min(WIN, N_NODES - base)


def _bucket_of(src, dst):
    return (src // WIN) * 4 + dst // WIN


def _chunk_plan(caps):
    """Cut buckets into chunks and pick a processing order.

    Returns a list of (bucket, offset_in_bucket, size, slot_base) with
    slot_base assigned in processing order. Order: smallest chunk first
    (fast pipeline fill), small chunks spread evenly among the big ones
    (their SWDGE descriptor-gen overhead hides under big transfers), and a
    small chunk last (short drain tail).
    """
    chunks = []
    for b in range(NBUCK):
        o = 0
        while o < caps[b]:
            sz = min(CHUNK, caps[b] - o)
            chunks.append((b, o, sz))
            o += sz
    chunks.sort(key=lambda c: -c[2])
    bigs = [c for c in chunks if c[2] >= CHUNK]
    smalls = [c for c in chunks if c[2] < CHUNK]
    seq = []
    first = smalls.pop() if smalls else (bigs.pop() if bigs else None)
    if first:
        seq.append(first)
    if PLAN_MODE == "interleave" and smalls and bigs:
        last = smalls.pop() if smalls else None
        stride = -(-len(bigs) // (len(smalls) + 1))
        bi = 0
        si = 0
        while bi < len(bigs):
            seq.extend(bigs[bi : bi + stride])
            bi += stride
            if si < len(smalls):
                seq.append(smalls[si])
                si += 1
        seq.extend(smalls[si:])
        if last:
            seq.append(last)
    else:
        seq.extend(bigs)
        seq.extend(smalls)
    # taper: re-split the trailing slots into 512-edge chunks so the drain
    # after the last big transfer is short
    tail = []
    acc = 0
    while seq and acc < TAPER and seq[-1][2] > TAPER_PIECE:
        b, o, sz = seq.pop()
        for i in range(0, sz, TAPER_PIECE):
            tail.append((b, o + i, min(TAPER_PIECE, sz - i)))
        acc += sz
    seq.extend(tail)
    plan = []
    base = 0
    for b, o, sz in seq:
        plan.append((b, o, sz, base))
        base += sz
    return plan


def _build_program(caps=None):
    if caps is None:
        caps = DEFAULT_CAPS
    caps = tuple(int(c) for c in caps)
    if caps in _NC_CACHE:
        return _NC_CACHE[caps]

    dt = mybir.dt
    AF = mybir.ActivationFunctionType
    ALU = mybir.AluOpType

    EP = sum(caps)
    TOT = EP // 128
    plan = _chunk_plan(caps)
    CMAX = max(sz for _, _, sz, _ in plan)

    nc = bacc.Bacc(
        "TRN2",
        target_bir_lowering=False,
        debug=False,
        enable_asserts=False,
        num_devices=NCORES,
    )
    emd = nc.dram_tensor("emd", [N_NODES, D], dt.bfloat16, kind="ExternalInput")
    idx_d = nc.dram_tensor("idx", [16, EP // 8], dt.int16, kind="ExternalInput")
    sel_d = nc.dram_tensor("sel", [16, 128], dt.bfloat16, kind="ExternalInput")
    w1_d = nc.dram_tensor("w1", [128, 512], dt.bfloat16, kind="ExternalInput")
    b1_d = nc.dram_tensor("b1", [128, 2], dt.float32, kind="ExternalInput")
    w2_d = nc.dram_tensor("w2", [128, 2], dt.bfloat16, kind="ExternalInput")
    b2_d = nc.dram_tensor("b2", [128, 1], dt.float32, kind="ExternalInput")
    out_d = nc.dram_tensor("out", [128, TOT], dt.float32, kind="ExternalOutput")

    with tile.TileContext(nc) as tc:
        with (
            tc.tile_pool(name="const", bufs=1) as cpool,
            tc.tile_pool(name="i", bufs=IBUFS) as ipool,
            tc.tile_pool(name="x", bufs=XBUFS) as xpool,
            tc.tile_pool(name="h", bufs=HBUFS) as hpool,
            tc.tile_pool(name="o", bufs=2) as opool,
            tc.tile_pool(name="ph", bufs=HPBUFS, space="PSUM") as php,
            tc.tile_pool(name="pl", bufs=2, space="PSUM") as plp,
            tc.tile_pool(name="pb", bufs=2, space="PSUM") as pbp,
        ):
            # index-path loads go first (the gather pipeline depends on
            # them); weight loads ride the Activation engine's HWDGE queue
            sel_sb = cpool.tile([16, 128], dt.bfloat16)
            nc.sync.dma_start(sel_sb[:, :], sel_d[:, :])
            # all (16-partition-wrapped) gather indices, loaded once; each
            # chunk broadcasts its slice to 128 partitions via a 0/1 matmul
            # on the raw bf16 bit patterns (exact, incl. subnormals)
            idx16_sb = cpool.tile([16, EP // 8], dt.int16)
            nc.sync.dma_start(idx16_sb[:, :], idx_d[:, :])
            w1_sb = cpool.tile([128, 512], dt.bfloat16)
            nc.scalar.dma_start(w1_sb[:, :], w1_d[:, :])
            b1_sb = cpool.tile([128, 2], dt.float32)
            nc.scalar.dma_start(b1_sb[:, :], b1_d[:, :])
            w2_sb = cpool.tile([128, 2], dt.bfloat16)
            nc.scalar.dma_start(w2_sb[:, :], w2_d[:, :])
            b2_sb = cpool.tile([128, 1], dt.float32)
            nc.scalar.dma_start(b2_sb[:, :], b2_d[:, :])

            # broadcast every chunk's indices to 128 partitions up front;
            # PE/DVE pipeline stays far ahead of the gathers consuming them
            sdis = []
            for k, (b, o, sz, base) in enumerate(plan):
                c8 = base // 8
                ibx = pbp.tile([128, CMAX // 8], dt.float32, tag="ibx")
                nc.tensor.matmul(
                    ibx[:, 0 : sz // 8], lhsT=sel_sb[:, :],
                    rhs=idx16_sb[:, c8 : c8 + sz // 8].bitcast(dt.bfloat16),
                    start=True, stop=True,
                )
                sdi = cpool.tile([128, sz // 8], dt.int16, name=f"sdi{k}")
                nc.vector.tensor_copy(
                    out=sdi[:, :].bitcast(dt.bfloat16),
                    in_=ibx[:, 0 : sz // 8],
                )
                sdis.append(sdi)

            for k, (b, o, sz, base) in enumerate(plan):
                ncols = sz // 128
                sb_, sl_ = _window(b >> 2)
                db_, dl_ = _window(b & 3)
                sdi = sdis[k]
                si = sdi[:, 0 : sz // 16]
                di = sdi[:, sz // 16 : sz // 8]
                xs = xpool.tile([128, CMAX], dt.bfloat16, tag="xs")
                xd = xpool.tile([128, CMAX], dt.bfloat16, tag="xd")
                nc.gpsimd.dma_gather(
                    xs[:, 0:sz].unsqueeze(1),
                    emd[sb_ : sb_ + sl_, :],
                    si,
                    sz,
                    sz,
                    D,
                    transpose=True,
                    single_packet=False,
                )
                nc.gpsimd.dma_gather(
                    xd[:, 0:sz].unsqueeze(1),
                    emd[db_ : db_ + dl_, :],
                    di,
                    sz,
                    sz,
                    D,
                    transpose=True,
                    single_packet=False,
                )

                lg = plp.tile([128, CMAX // 128], dt.float32, tag="lg")
                ntile = (sz + 511) // 512
                # software pipeline: L2 of tile t issues L2LAG tiles late
                hq = []
                for t in range(ntile + L2LAG):
                    if t < ntile:
                        e0 = t * 512
                        n = min(512, sz - e0)
                        h0p = php.tile([128, 512], dt.float32, tag="h0p")
                        h1p = php.tile([128, 512], dt.float32, tag="h1p")
                        nc.tensor.matmul(
                            h0p[:, 0:n], lhsT=w1_sb[:, 0:128],
                            rhs=xs[:, e0 : e0 + n], start=True, stop=False,
                        )
                        nc.tensor.matmul(
                            h0p[:, 0:n], lhsT=w1_sb[:, 256:384],
                            rhs=xd[:, e0 : e0 + n], start=False, stop=True,
                        )
                        nc.tensor.matmul(
                            h1p[:, 0:n], lhsT=w1_sb[:, 128:256],
                            rhs=xs[:, e0 : e0 + n], start=True, stop=False,
                        )
                        nc.tensor.matmul(
                            h1p[:, 0:n], lhsT=w1_sb[:, 384:512],
                            rhs=xd[:, e0 : e0 + n], start=False, stop=True,
                        )
                        h0s = hpool.tile([128, 512], dt.bfloat16, tag="h0s")
                        h1s = hpool.tile([128, 512], dt.bfloat16, tag="h1s")
                        nc.scalar.activation(
                            h0s[:, 0:n], h0p[:, 0:n], AF.Relu, bias=b1_sb[:, 0:1]
                        )
                        nc.vector.tensor_scalar(
                            h1s[:, 0:n], h1p[:, 0:n],
                            b1_sb[:, 1:2], 0.0, ALU.add, ALU.max,
                        )
                        hq.append((t, n, h0s, h1s))
                    if t >= L2LAG:
                        pt, pn, p0, p1 = hq[t - L2LAG]
                        for s in range((pn + 127) // 128):
                            ns = min(128, pn - s * 128)
                            col = pt * 4 + s
                            nc.tensor.matmul(
                                lg[0:ns, col : col + 1],
                                lhsT=p0[:, s * 128 : s * 128 + ns],
                                rhs=w2_sb[:, 0:1], start=True, stop=False,
                            )
                            nc.tensor.matmul(
                                lg[0:ns, col : col + 1],
                                lhsT=p1[:, s * 128 : s * 128 + ns],
                                rhs=w2_sb[:, 1:2], start=False, stop=True,
                            )
                sig = opool.tile([128, CMAX // 128], dt.float32, tag="sig")
                nc.scalar.activation(
                    sig[:, 0:ncols], lg[:, 0:ncols], AF.Sigmoid, bias=b2_sb[:, 0:1]
                )
                nc.sync.dma_start(
                    out_d[:, base // 128 : base // 128 + ncols], sig[:, 0:ncols]
                )

    nc.compile()
    _NC_CACHE[caps] = nc
    return nc


def _wrap_idx(vals):
    """int16 [n] -> [16, n//16] wrapped in 16 partitions."""
    n = vals.shape[0]
    return np.ascontiguousarray(vals.reshape(n // 16, 16).T)


def _prepare_core(ei_core, caps, plan):
    """Bucket + chunk one core's edges into slot order.

    Returns (idx_combined, edge_of_slot): idx_combined is [16, EP//8] int16
    holding, per chunk, the wrapped src indices then the wrapped dst indices;
    edge_of_slot maps slot -> local edge id (or -1 for padding).
    """
    src = ei_core[:, 0].astype(np.int64)
    dst = ei_core[:, 1].astype(np.int64)
    bucket = _bucket_of(src, dst)
    order = np.argsort(bucket, kind="stable")
    counts = np.bincount(bucket, minlength=NBUCK)
    starts = np.zeros(NBUCK + 1, np.int64)
    np.cumsum(counts, out=starts[1:])

    EP = sum(caps)
    idx = np.zeros((16, EP // 8), np.int16)
    edge_of_slot = np.full(EP, -1, np.int64)
    for b, o, sz, base in plan:
        nb = int(counts[b])
        lo = min(o, nb)
        hi = min(o + sz, nb)
        sc = np.zeros(sz, np.int16)
        dc = np.zeros(sz, np.int16)
        if hi > lo:
            sel = order[starts[b] + lo : starts[b] + hi]
            sc[: hi - lo] = (src[sel] - (b >> 2) * WIN).astype(np.int16)
            dc[: hi - lo] = (dst[sel] - (b & 3) * WIN).astype(np.int16)
            edge_of_slot[base : base + hi - lo] = sel
        c8 = base // 8
        idx[:, c8 : c8 + sz // 16] = _wrap_idx(sc)
        idx[:, c8 + sz // 16 : c8 + sz // 8] = _wrap_idx(dc)
    return idx, edge_of_slot


def kernel(emd_all, edge_index, W1, b1, W2, b2):
    global LAST_RESULTS
    emd_bf = np.ascontiguousarray(np.asarray(emd_all, dtype=np.float32)).astype(BF16)
    ei = np.asarray(edge_index).astype(np.int64)
    W1 = np.asarray(W1, dtype=np.float32)
    W2 = np.asarray(W2, dtype=np.float32)
    b1 = np.asarray(b1, dtype=np.float32).reshape(-1)
    b2 = np.asarray(b2, dtype=np.float32).reshape(-1)

    # Re-shard edges across cores so each core gets an equal slice of every
    # bucket: per-core bucket counts become ceil(n_b/8), which minimises the
    # shared static capacity padding.
    bk_all = _bucket_of(ei[:, 0].astype(np.int64), ei[:, 1].astype(np.int64))
    gorder = np.argsort(bk_all, kind="stable")
    gcounts = np.bincount(bk_all, minlength=NBUCK)
    core_of_edge = np.empty(E_TOTAL, np.int64)
    pos = 0
    per_core_n = np.zeros(NCORES, np.int64)
    for b in range(NBUCK):
        nb = int(gcounts[b])
        sel = gorder[pos : pos + nb]
        # deal bucket b round-robin-in-blocks across cores
        q, r = divmod(nb, NCORES)
        o = 0
        for c in range(NCORES):
            take = q + (1 if c < r else 0)
            core_of_edge[sel[o : o + take]] = c
            per_core_n[c] += take
            o += take
        pos += nb
    caps_counts = np.zeros((NCORES, NBUCK), np.int64)
    edges_of_core = [np.where(core_of_edge == c)[0] for c in range(NCORES)]
    for c in range(NCORES):
        bk = bk_all[edges_of_core[c]]
        caps_counts[c] = np.bincount(bk, minlength=NBUCK)
    caps = tuple(
        int(max(128, -(-int(caps_counts[:, b].max()) // 128) * 128))
        for b in range(NBUCK)
    )
    plan = _chunk_plan(caps)

    # lhsT blocks: [src->h0, src->h1, dst->h0, dst->h1]
    w1_arr = np.concatenate(
        [W1[:D, :D], W1[:D, D:], W1[D:, :D], W1[D:, D:]], axis=1
    ).astype(BF16)
    b1_arr = np.ascontiguousarray(np.stack([b1[:128], b1[128:]], axis=1))
    w2_arr = np.ascontiguousarray(np.stack([W2[:128, 0], W2[128:, 0]], axis=1)).astype(
        BF16
    )
    b2_arr = np.full((128, 1), b2[0], np.float32)
    sel_arr = np.zeros((16, 128), np.float32)
    sel_arr[np.arange(128) % 16, np.arange(128)] = 1.0
    sel_arr = sel_arr.astype(BF16)

    in_maps = []
    unshard = []
    for c in range(NCORES):
        idx, edge_of_slot = _prepare_core(ei[edges_of_core[c]], caps, plan)
        unshard.append(edge_of_slot)
        in_maps.append(
            {
                "emd": emd_bf,
                "idx": idx,
                "sel": sel_arr,
                "w1": w1_arr,
                "b1": b1_arr,
                "w2": w2_arr,
                "b2": b2_arr,
            }
        )

    nc = _build_program(caps)
    res = run_bass_kernel_spmd(nc, in_maps, core_ids=list(range(NCORES)))
    LAST_RESULTS = res

    y = np.empty((E_TOTAL,), np.float32)
    for c in range(NCORES):
        edge_of_slot = unshard[c]  # slot -> index into edges_of_core[c]
        out = np.asarray(res.results[c]["out"], dtype=np.float32)  # [128, TOT]
        flat = out.T.reshape(-1)  # slot-ordered
        mask = edge_of_slot >= 0
        y[edges_of_core[c][edge_of_slot[mask]]] = flat[mask]
    return y.reshape(E_TOTAL, 1)


if __name__ == "__main__":
    rng = np.random.default_rng(0)
    emd = rng.standard_normal((N_NODES, D), dtype=np.float32)
    ei = rng.integers(0, N_NODES, size=(E_TOTAL, 2)).astype(np.int32)
    W1 = rng.standard_normal((2 * D, H), dtype=np.float32) / np.sqrt(2 * D)
    W2 = rng.standard_normal((H, 1), dtype=np.float32) / np.sqrt(H)
    out = kernel(emd, ei, W1, np.zeros(H, np.float32), W2, np.zeros(1, np.float32))
    print(out.shape, out[:4, 0])


# revision 31
# speedup vs baseline: 5.3182x; 1.0019x over previous
"""Link-predictor GNN kernel for 8 TRN2 NeuronCores.

Strategy (per sharding hint): shard edges across 8 cores (data parallel),
replicate the bf16 node-embedding table + MLP weights on every core.

Edges are bucketed by (src_window, dst_window) where a window is 25000
table rows (4 windows cover 100000 nodes) so window-relative node ids fit
the int16 indices of the batched SWDGE dma_gather. Each bucket is dealt
evenly across the 8 cores (so the shared static per-bucket capacity padding
is minimal) and cut into <=CHUNK-edge chunks for pipelining.

Per chunk: one dma_gather per side (transpose=True, single_packet=False)
lands embeddings directly in X^T layout [128 dims, n_edges] in SBUF — no PE
transposes needed, and the ~1us SWDGE fixed overhead is amortized over
thousands of rows instead of the 128 an indirect_dma_start moves. The
16-partition-wrapped index arrays are loaded once in compact [16, cols]
form and replicated to the 128 partitions the gather hardware expects via
0/1-matmul broadcasts of the raw bf16 bit patterns (bit-exact, subnormals
included), cutting idx DMA traffic 8x.

MLP per 512-edge tile: h = relu(W1s^T Xs + W1d^T Xd + b1) via 4 matmuls
accumulating in PSUM; relu of h-half-0 on ACT (bias fused), half-1 on DVE
(tensor_scalar add+max). Layer 2 contracts h against W2 using h-subtiles
as the stationary operand: 2 matmuls of N=1 per 128-edge subtile writing
one PSUM column [128 edges, 1] (Ldweights is free in the cost model); a
whole chunk's logits accumulate into one PSUM tile so a single sigmoid +
one small DMA per chunk emits [128, cols] f32 results. Host inverts the
slot permutation.
"""

import sys

sys.path.insert(0, "/opt/trn_rl_repo")

import numpy as np
import ml_dtypes

from concourse import bacc, mybir, tile
from concourse.bass_utils import run_bass_kernel_spmd

BF16 = ml_dtypes.bfloat16

N_NODES = 100000
D = 128
H = 256
E_TOTAL = 600000
NCORES = 8
E_CORE = 75000
WIN = 25000                      # table-row window (< 2^15 for int16 idx)
NBUCK = 16                       # 4 src windows x 4 dst windows
CHUNK = 2944                     # max edges per gather/compute chunk
XBUFS = 4                        # gather buffer depth per side
IBUFS = 4                        # idx tile depth
HPBUFS = 2                       # PSUM h depth
L2LAG = 1                        # tiles of lag between L1 and L2 issue
HBUFS = 3                        # h sbuf tile depth
PLAN_MODE = "smalls_last"        # or "interleave"
TAPER = 6000                     # trailing slots re-split into smaller chunks
TAPER_PIECE = 1024               # taper piece size (multiple of 128)
RAMP = 0                         # 512-edge pieces peeled off the first chunk

# Bucket capacities for the canonical setup_inputs() edge set (max count over
# the 8 cores per bucket, rounded up to 128). kernel() recomputes these from
# its actual inputs; this default only serves _build_program() callers that
# have no inputs (e.g. a standalone TimelineSim of the program).
DEFAULT_CAPS = (4736, 4736, 4736, 4864, 4736, 4736, 4736, 4736,
                4736, 4736, 4736, 4736, 4736, 4736, 4736, 4736)

LAST_RESULTS = None
_NC_CACHE: dict = {}


def _window(w):
    base = w * WIN
    return base, min(WIN, N_NODES - base)


def _bucket_of(src, dst):
    return (src // WIN) * 4 + dst // WIN


def _chunk_plan(caps):
    """Cut buckets into chunks and pick a processing order.

    Returns a list of (bucket, offset_in_bucket, size, slot_base) with
    slot_base assigned in processing order. Order: smallest chunk first
    (fast pipeline fill), small chunks spread evenly among the big ones
    (their SWDGE descriptor-gen overhead hides under big transfers), and a
    small chunk last (short drain tail).
    """
    chunks = []
    for b in range(NBUCK):
        o = 0
        while o < caps[b]:
            sz = min(CHUNK, caps[b] - o)
            chunks.append((b, o, sz))
            o += sz
    chunks.sort(key=lambda c: -c[2])
    bigs = [c for c in chunks if c[2] >= CHUNK]
    smalls = [c for c in chunks if c[2] < CHUNK]
    seq = []
    first = smalls.pop() if smalls else (bigs.pop() if bigs else None)
    if first:
        seq.append(first)
    if PLAN_MODE == "interleave" and smalls and bigs:
        last = smalls.pop() if smalls else None
        stride = -(-len(bigs) // (len(smalls) + 1))
        bi = 0
        si = 0
        while bi < len(bigs):
            seq.extend(bigs[bi : bi + stride])
            bi += stride
            if si < len(smalls):
                seq.append(smalls[si])
                si += 1
        seq.extend(smalls[si:])
        if last:
            seq.append(last)
    else:
        seq.extend(bigs)
        seq.extend(smalls)
    # ramp: peel small pieces off the front so the first gather's gen and
    # transfer are short and the pipeline fills fast
    for _ in range(RAMP):
        if seq and seq[0][2] > 512:
            b, o, sz = seq.pop(0)
            seq.insert(0, (b, o + 512, sz - 512))
            seq.insert(0, (b, o, 512))
    # taper: re-split the trailing slots into 512-edge chunks so the drain
    # after the last big transfer is short
    tail = []
    acc = 0
    while seq and acc < TAPER and seq[-1][2] > TAPER_PIECE:
        b, o, sz = seq.pop()
        for i in range(0, sz, TAPER_PIECE):
            tail.append((b, o + i, min(TAPER_PIECE, sz - i)))
        acc += sz
    seq.extend(tail)
    plan = []
    base = 0
    for b, o, sz in seq:
        plan.append((b, o, sz, base))
        base += sz
    return plan


def _build_program(caps=None):
    if caps is None:
        caps = DEFAULT_CAPS
    caps = tuple(int(c) for c in caps)
    if caps in _NC_CACHE:
        return _NC_CACHE[caps]

    dt = mybir.dt
    AF = mybir.ActivationFunctionType
    ALU = mybir.AluOpType

    EP = sum(caps)
    TOT = EP // 128
    plan = _chunk_plan(caps)
    CMAX = max(sz for _, _, sz, _ in plan)

    nc = bacc.Bacc(
        "TRN2",
        target_bir_lowering=False,
        debug=False,
        enable_asserts=False,
        num_devices=NCORES,
    )
    emd = nc.dram_tensor("emd", [N_NODES, D], dt.bfloat16, kind="ExternalInput")
    idx_d = nc.dram_tensor("idx", [16, EP // 8], dt.int16, kind="ExternalInput")
    sel_d = nc.dram_tensor("sel", [16, 128], dt.bfloat16, kind="ExternalInput")
    w1_d = nc.dram_tensor("w1", [128, 512], dt.bfloat16, kind="ExternalInput")
    b1_d = nc.dram_tensor("b1", [128, 2], dt.float32, kind="ExternalInput")
    w2_d = nc.dram_tensor("w2", [128, 2], dt.bfloat16, kind="ExternalInput")
    b2_d = nc.dram_tensor("b2", [128, 1], dt.float32, kind="ExternalInput")
    out_d = nc.dram_tensor("out", [128, TOT], dt.float32, kind="ExternalOutput")

    with tile.TileContext(nc) as tc:
        with (
            tc.tile_pool(name="const", bufs=1) as cpool,
            tc.tile_pool(name="i", bufs=IBUFS) as ipool,
            tc.tile_pool(name="x", bufs=XBUFS) as xpool,
            tc.tile_pool(name="h", bufs=HBUFS) as hpool,
            tc.tile_pool(name="o", bufs=2) as opool,
            tc.tile_pool(name="ph", bufs=HPBUFS, space="PSUM") as php,
            tc.tile_pool(name="pl", bufs=2, space="PSUM") as plp,
            tc.tile_pool(name="pb", bufs=2, space="PSUM") as pbp,
        ):
            # index-path loads go first (the gather pipeline depends on
            # them); weight loads ride the Activation engine's HWDGE queue
            sel_sb = cpool.tile([16, 128], dt.bfloat16)
            nc.sync.dma_start(sel_sb[:, :], sel_d[:, :])
            # all (16-partition-wrapped) gather indices, loaded once; each
            # chunk broadcasts its slice to 128 partitions via a 0/1 matmul
            # on the raw bf16 bit patterns (exact, incl. subnormals)
            idx16_sb = cpool.tile([16, EP // 8], dt.int16)
            nc.sync.dma_start(idx16_sb[:, :], idx_d[:, :])
            w1_sb = cpool.tile([128, 512], dt.bfloat16)
            nc.scalar.dma_start(w1_sb[:, :], w1_d[:, :])
            b1_sb = cpool.tile([128, 2], dt.float32)
            nc.scalar.dma_start(b1_sb[:, :], b1_d[:, :])
            w2_sb = cpool.tile([128, 2], dt.bfloat16)
            nc.scalar.dma_start(w2_sb[:, :], w2_d[:, :])
            b2_sb = cpool.tile([128, 1], dt.float32)
            nc.scalar.dma_start(b2_sb[:, :], b2_d[:, :])

            # broadcast every chunk's indices to 128 partitions up front;
            # PE/DVE pipeline stays far ahead of the gathers consuming them
            sdis = []
            for k, (b, o, sz, base) in enumerate(plan):
                c8 = base // 8
                ibx = pbp.tile([128, CMAX // 8], dt.float32, tag="ibx")
                nc.tensor.matmul(
                    ibx[:, 0 : sz // 8], lhsT=sel_sb[:, :],
                    rhs=idx16_sb[:, c8 : c8 + sz // 8].bitcast(dt.bfloat16),
                    start=True, stop=True,
                )
                sdi = cpool.tile([128, sz // 8], dt.int16, name=f"sdi{k}")
                nc.vector.tensor_copy(
                    out=sdi[:, :].bitcast(dt.bfloat16),
                    in_=ibx[:, 0 : sz // 8],
                )
                sdis.append(sdi)

            for k, (b, o, sz, base) in enumerate(plan):
                ncols = sz // 128
                sb_, sl_ = _window(b >> 2)
                db_, dl_ = _window(b & 3)
                sdi = sdis[k]
                si = sdi[:, 0 : sz // 16]
                di = sdi[:, sz // 16 : sz // 8]
                xs = xpool.tile([128, CMAX], dt.bfloat16, tag="xs")
                xd = xpool.tile([128, CMAX], dt.bfloat16, tag="xd")
                nc.gpsimd.dma_gather(
                    xs[:, 0:sz].unsqueeze(1),
                    emd[sb_ : sb_ + sl_, :],
                    si,
                    sz,
                    sz,
                    D,
                    transpose=True,
                    single_packet=False,
                )
                nc.gpsimd.dma_gather(
                    xd[:, 0:sz].unsqueeze(1),
                    emd[db_ : db_ + dl_, :],
                    di,
                    sz,
                    sz,
                    D,
                    transpose=True,
                    single_packet=False,
                )

                lg = plp.tile([128, CMAX // 128], dt.float32, tag="lg")
                ntile = (sz + 511) // 512
                # software pipeline: L2 of tile t issues L2LAG tiles late
                hq = []
                for t in range(ntile + L2LAG):
                    if t < ntile:
                        e0 = t * 512
                        n = min(512, sz - e0)
                        h0p = php.tile([128, 512], dt.float32, tag="h0p")
                        h1p = php.tile([128, 512], dt.float32, tag="h1p")
                        nc.tensor.matmul(
                            h0p[:, 0:n], lhsT=w1_sb[:, 0:128],
                            rhs=xs[:, e0 : e0 + n], start=True, stop=False,
                        )
                        nc.tensor.matmul(
                            h0p[:, 0:n], lhsT=w1_sb[:, 256:384],
                            rhs=xd[:, e0 : e0 + n], start=False, stop=True,
                        )
                        nc.tensor.matmul(
                            h1p[:, 0:n], lhsT=w1_sb[:, 128:256],
                            rhs=xs[:, e0 : e0 + n], start=True, stop=False,
                        )
                        nc.tensor.matmul(
                            h1p[:, 0:n], lhsT=w1_sb[:, 384:512],
                            rhs=xd[:, e0 : e0 + n], start=False, stop=True,
                        )
                        h0s = hpool.tile([128, 512], dt.bfloat16, tag="h0s")
                        h1s = hpool.tile([128, 512], dt.bfloat16, tag="h1s")
                        nc.scalar.activation(
                            h0s[:, 0:n], h0p[:, 0:n], AF.Relu, bias=b1_sb[:, 0:1]
                        )
                        nc.vector.tensor_scalar(
                            h1s[:, 0:n], h1p[:, 0:n],
                            b1_sb[:, 1:2], 0.0, ALU.add, ALU.max,
                        )
                        hq.append((t, n, h0s, h1s))
                    if t >= L2LAG:
                        pt, pn, p0, p1 = hq[t - L2LAG]
                        for s in range((pn + 127) // 128):
                            ns = min(128, pn - s * 128)
                            col = pt * 4 + s
                            nc.tensor.matmul(
                                lg[0:ns, col : col + 1],
                                lhsT=p0[:, s * 128 : s * 128 + ns],
                                rhs=w2_sb[:, 0:1], start=True, stop=False,
                            )
                            nc.tensor.matmul(
                                lg[0:ns, col : col + 1],
                                lhsT=p1[:, s * 128 : s * 128 + ns],
                                rhs=w2_sb[:, 1:2], start=False, stop=True,
                            )
                sig = opool.tile([128, CMAX // 128], dt.float32, tag="sig")
                nc.scalar.activation(
                    sig[:, 0:ncols], lg[:, 0:ncols], AF.Sigmoid, bias=b2_sb[:, 0:1]
                )
                nc.sync.dma_start(
                    out_d[:, base // 128 : base // 128 + ncols], sig[:, 0:ncols]
                )

    nc.compile()
    _NC_CACHE[caps] = nc
    return nc


def _wrap_idx(vals):
    """int16 [n] -> [16, n//16] wrapped in 16 partitions."""
    n = vals.shape[0]
    return np.ascontiguousarray(vals.reshape(n // 16, 16).T)


def _prepare_core(ei_core, caps, plan):
    """Bucket + chunk one core's edges into slot order.

    Returns (idx_combined, edge_of_slot): idx_combined is [16, EP//8] int16
    holding, per chunk, the wrapped src indices then the wrapped dst indices;
    edge_of_slot maps slot -> local edge id (or -1 for padding).
    """
    src = ei_core[:, 0].astype(np.int64)
    dst = ei_core[:, 1].astype(np.int64)
    bucket = _bucket_of(src, dst)
    order = np.argsort(bucket, kind="stable")
    counts = np.bincount(bucket, minlength=NBUCK)
    starts = np.zeros(NBUCK + 1, np.int64)
    np.cumsum(counts, out=starts[1:])

    EP = sum(caps)
    idx = np.zeros((16, EP // 8), np.int16)
    edge_of_slot = np.full(EP, -1, np.int64)
    for b, o, sz, base in plan:
        nb = int(counts[b])
        lo = min(o, nb)
        hi = min(o + sz, nb)
        sc = np.zeros(sz, np.int16)
        dc = np.zeros(sz, np.int16)
        if hi > lo:
            sel = order[starts[b] + lo : starts[b] + hi]
            sc[: hi - lo] = (src[sel] - (b >> 2) * WIN).astype(np.int16)
            dc[: hi - lo] = (dst[sel] - (b & 3) * WIN).astype(np.int16)
            edge_of_slot[base : base + hi - lo] = sel
        c8 = base // 8
        idx[:, c8 : c8 + sz // 16] = _wrap_idx(sc)
        idx[:, c8 + sz // 16 : c8 + sz // 8] = _wrap_idx(dc)
    return idx, edge_of_slot


def kernel(emd_all, edge_index, W1, b1, W2, b2):
    global LAST_RESULTS
    emd_bf = np.ascontiguousarray(np.asarray(emd_all, dtype=np.float32)).astype(BF16)
    ei = np.asarray(edge_index).astype(np.int64)
    W1 = np.asarray(W1, dtype=np.float32)
    W2 = np.asarray(W2, dtype=np.float32)
    b1 = np.asarray(b1, dtype=np.float32).reshape(-1)
    b2 = np.asarray(b2, dtype=np.float32).reshape(-1)

    # Re-shard edges across cores so each core gets an equal slice of every
    # bucket: per-core bucket counts become ceil(n_b/8), which minimises the
    # shared static capacity padding.
    bk_all = _bucket_of(ei[:, 0].astype(np.int64), ei[:, 1].astype(np.int64))
    gorder = np.argsort(bk_all, kind="stable")
    gcounts = np.bincount(bk_all, minlength=NBUCK)
    core_of_edge = np.empty(E_TOTAL, np.int64)
    pos = 0
    per_core_n = np.zeros(NCORES, np.int64)
    for b in range(NBUCK):
        nb = int(gcounts[b])
        sel = gorder[pos : pos + nb]
        # deal bucket b round-robin-in-blocks across cores
        q, r = divmod(nb, NCORES)
        o = 0
        for c in range(NCORES):
            take = q + (1 if c < r else 0)
            core_of_edge[sel[o : o + take]] = c
            per_core_n[c] += take
            o += take
        pos += nb
    caps_counts = np.zeros((NCORES, NBUCK), np.int64)
    edges_of_core = [np.where(core_of_edge == c)[0] for c in range(NCORES)]
    for c in range(NCORES):
        bk = bk_all[edges_of_core[c]]
        caps_counts[c] = np.bincount(bk, minlength=NBUCK)
    caps = tuple(
        int(max(128, -(-int(caps_counts[:, b].max()) // 128) * 128))
        for b in range(NBUCK)
    )
    plan = _chunk_plan(caps)

    # lhsT blocks: [src->h0, src->h1, dst->h0, dst->h1]
    w1_arr = np.concatenate(
        [W1[:D, :D], W1[:D, D:], W1[D:, :D], W1[D:, D:]], axis=1
    ).astype(BF16)
    b1_arr = np.ascontiguousarray(np.stack([b1[:128], b1[128:]], axis=1))
    w2_arr = np.ascontiguousarray(np.stack([W2[:128, 0], W2[128:, 0]], axis=1)).astype(
        BF16
    )
    b2_arr = np.full((128, 1), b2[0], np.float32)
    sel_arr = np.zeros((16, 128), np.float32)
    sel_arr[np.arange(128) % 16, np.arange(128)] = 1.0
    sel_arr = sel_arr.astype(BF16)

    in_maps = []
    unshard = []
    for c in range(NCORES):
        idx, edge_of_slot = _prepare_core(ei[edges_of_core[c]], caps, plan)
        unshard.append(edge_of_slot)
        in_maps.append(
            {
                "emd": emd_bf,
                "idx": idx,
                "sel": sel_arr,
                "w1": w1_arr,
                "b1": b1_arr,
                "w2": w2_arr,
                "b2": b2_arr,
            }
        )

    nc = _build_program(caps)
    res = run_bass_kernel_spmd(nc, in_maps, core_ids=list(range(NCORES)))
    LAST_RESULTS = res

    y = np.empty((E_TOTAL,), np.float32)
    for c in range(NCORES):
        edge_of_slot = unshard[c]  # slot -> index into edges_of_core[c]
        out = np.asarray(res.results[c]["out"], dtype=np.float32)  # [128, TOT]
        flat = out.T.reshape(-1)  # slot-ordered
        mask = edge_of_slot >= 0
        y[edges_of_core[c][edge_of_slot[mask]]] = flat[mask]
    return y.reshape(E_TOTAL, 1)


if __name__ == "__main__":
    rng = np.random.default_rng(0)
    emd = rng.standard_normal((N_NODES, D), dtype=np.float32)
    ei = rng.integers(0, N_NODES, size=(E_TOTAL, 2)).astype(np.int32)
    W1 = rng.standard_normal((2 * D, H), dtype=np.float32) / np.sqrt(2 * D)
    W2 = rng.standard_normal((H, 1), dtype=np.float32) / np.sqrt(H)
    out = kernel(emd, ei, W1, np.zeros(H, np.float32), W2, np.zeros(1, np.float32))
    print(out.shape, out[:4, 0])
